# revision 26
# baseline (speedup 1.0000x reference)
"""Trainium2 Bass kernel for the 4-kernel MMD permutation test (nn_DUAL_78237124264373).

Sharding: 8 cores = 2 kernel-pairs x 4 permutation quarters. Core c<4 computes
kernels (0,1) [gaussian, laplacian] for perms [50*(c%4), 50*(c%4)+50); core
c>=4 the same for kernels (2,3). The host merges the [2, 1+50] per-core
outputs, so each core only ever evaluates TWO kernel matrices and the
activation-table sequence is exactly EXP (pre-warmed) -> SQRT -> EXP: the
swap points are pinned with zero-valued bias/scale tokens that data-depend
on the previous block's last op, so the Tile scheduler cannot interleave.

Per-core pipeline (slot a = gaussian, slot b = laplacian):
  d2 = L^T R on the PE in f32r (L = [Zt; 1], R = [-2 Zt; sq + B]), 12 PSUM
  pieces (6 row tiles x 512+256) in a 4-deep PSUM pool. Slot-a K = exp(ga*d2)
  straight out of PSUM with its M0 = A_aug K chunk matmul right behind, while
  the DVE lands clamped d2 in SBUF for the sqrt block. After the swap back to
  EXP, slot-b K = exp(lb*dist) runs chunk-by-chunk with M0 trailing. Each
  slot's row stats (aKa, aK1, colA), U_b vector and PE transpose into the
  partition-0 assembly row run as soon as that slot's M0 completes, so slot
  a's tail hides under the SQRT/slot-b window. U_b = KAP*(aKa - aK1) +
  W_corr @ e + (2/c2)*t + ck, with pair sums t reduced from host-gathered
  16-wide partial squares and e the K0[j, 384+j] stripe (3 extra pair
  blocks). The final scalar assembly (U, ck) happens on partition 0.
"""

import sys

import numpy as np

if "/opt/trn_rl_repo" not in sys.path:
    sys.path.insert(0, "/opt/trn_rl_repo")

import ml_dtypes

import concourse.bacc as bacc
import concourse.bass as bass
import concourse.mybir as mybir
import concourse.tile as tile
from concourse import bass_utils

N = 384
NM = 768
D = 64
NPER = 200
NC = 8
PPC = 50                      # perms per core
ROWS = PPC + 2                # + X-identity + Y-identity rows
NBLK = 3 * PPC + 3            # pair blocks of 128: 50 perms x 3 + stripe x 3
BIAS = 1e-3                   # keeps d2 > 0 under f32r rounding (see d0c)
C1 = float(N * (N - 1))
C2 = float(N * N)
KAP = np.float32(2.0 / C1 + 2.0 / C2)
CB1 = np.float32(1.0 / C1 + 2.0 / C2)
CB2 = np.float32(1.0 / C1)
TCO = np.float32(2.0 / C2)
IC1 = np.float32(1.0 / C1)
IC2 = np.float32(1.0 / C2)

F32 = mybir.dt.float32
F32R = mybir.dt.float32r
BF16 = mybir.dt.bfloat16
AF = mybir.ActivationFunctionType
ALU = mybir.AluOpType


def _build():
    nc = bacc.Bacc("TRN2", target_bir_lowering=False, debug=False)
    with tile.TileContext(nc) as tc:
        with tc.tile_pool(name="dram", bufs=1, space="DRAM") as dram, \
             tc.tile_pool(name="io", bufs=1) as io, \
             tc.tile_pool(name="big", bufs=1) as big, \
             tc.tile_pool(name="scr", bufs=1) as scr, \
             tc.tile_pool(name="sml", bufs=1) as sml:

            def din(name, shape, dt=F32):
                return dram.tile(shape, dt, kind="ExternalInput", name=name,
                                 uniquify=False)

            zlr_d = din("zlr", [D + 1, 2 * NM], F32R)
            psq_d = din("psq", [128, NBLK * 4], BF16)
            bfp_d = din("bfp", [128, NM + 6 * D + 3 * D], BF16)
            fsp_d = din("fsp", [128, 32], F32)
            idm_d = din("idm", [128, 64], F32)
            out_d = dram.tile([2, 1 + PPC], F32, kind="ExternalOutput",
                              name="out", uniquify=False)

            # ---- input DMAs; zlr is [R | L] so the first piece lands first
            zlr = io.tile([D + 1, 2 * NM], F32R, name="zlr_sb")
            nc.sync.dma_start(out=zlr[:, 0:896], in_=zlr_d[:, 0:896])
            nc.sync.dma_start(out=zlr[:, 896:], in_=zlr_d[:, 896:])
            psq = io.tile([128, NBLK * 4], BF16, name="psq_sb")
            nc.sync.dma_start(out=psq[:], in_=psq_d[:])
            bfp = io.tile([128, NM + 6 * D + 3 * D], BF16, name="bfp_sb")
            nc.sync.dma_start(out=bfp[:], in_=bfp_d[:])
            fsp = io.tile([128, 32], F32, name="fsp_sb")
            nc.sync.dma_start(out=fsp[:], in_=fsp_d[:])
            idm = io.tile([128, 64], F32, name="idm_sb")
            nc.sync.dma_start(out=idm[:], in_=idm_d[:])

            zr = zlr[:, 0:NM]
            zl = zlr[:, NM:2 * NM]
            astk = bfp[:, 0:NM]                      # A rows at 0-51 / 64-115
            atp = bfp[:, NM:NM + 6 * D]              # A^T chunks, 64-padded
            wct = bfp[:, NM + 6 * D:NM + 9 * D]      # W_corr^T chunks
            sqc = fsp[:, 0:6]                        # sq columns per row tile
            gbias = fsp[:, 6:12]                     # ga * sq per row tile
            ga = fsp[:, 12:13]
            lb = fsp[:, 13:14]
            zero = fsp[:, 14:15]
            aux4i = fsp[0:1, 16:18]                  # 768*d0c*IC1 per slot

            ones = io.tile([128, 1], F32, name="ones_sb")
            nc.vector.memset(ones[:], 1.0)
            onesb = io.tile([128, 1], BF16, name="onesb_sb")
            nc.vector.memset(onesb[:], 1.0)

            d2sb = big.tile([128, 6 * NM], F32, name="d2sb")
            dist = big.tile([128, 6 * NM], F32, name="dist_sb")
            kta = big.tile([128, 6 * NM], BF16, name="kta")
            ktb = big.tile([128, 6 * NM], BF16, name="ktb")
            M0sb = big.tile([128, NM], F32, name="M0sb")
            sA = scr.tile([128, N], F32, name="sA")
            sB = scr.tile([128, NM], F32, name="sB")
            pair2 = sml.tile([128, NBLK], F32, name="pair2")
            arow = sml.tile([128, 1], F32, name="arow")
            colA = sml.tile([128, 1], F32, name="colA")
            q0c = sml.tile([128, 1], F32, name="q0c")
            pack = sml.tile([128, 4], F32, name="pack")

            # warm the EXP activation table while DMAs are in flight
            warm = sml.tile([128, 1], F32, name="warm")
            nc.scalar.activation(warm[0:1, :], ones[0:1, :], AF.Exp,
                                 bias=0.0, scale=1.0)

            # ---- pair d2: reduce the host 16-wide partial squares ----
            psq3 = psq.rearrange("p (b d) -> p b d", d=4)
            nc.vector.tensor_reduce(pair2[:], psq3[:],
                                    axis=mybir.AxisListType.X, op=ALU.add)

            # ---- d2 phase: f32r matmuls, one [128,768] PSUM tile per row
            # tile, 3 deep; slot-a exp from PSUM; DVE lands d2 for sqrt ----
            with tc.tile_pool(name="psA", bufs=3, space="PSUM") as psA:
                for r in range(6):
                    lhs = zl[:, 128 * r:128 * (r + 1)]
                    ps_d2 = psA.tile([128, NM], F32, tag="d2",
                                     name=f"ps_d2_{r}")
                    nc.tensor.matmul(ps_d2[:, 0:512], lhs, zr[:, 0:512],
                                     start=True, stop=True)
                    nc.tensor.matmul(ps_d2[:, 512:NM], lhs, zr[:, 512:NM],
                                     start=True, stop=True)
                    sl = slice(NM * r, NM * (r + 1))
                    nc.scalar.activation(kta[:, sl], ps_d2[:], AF.Exp,
                                         scale=ga, bias=gbias[:, r:r + 1])
                    nc.vector.tensor_scalar(
                        out=d2sb[:, sl], in0=ps_d2[:],
                        scalar1=sqc[:, r:r + 1], scalar2=0.0,
                        op0=ALU.add, op1=ALU.max)


            with tc.tile_pool(name="psB", bufs=1, space="PSUM") as psB, \
                 tc.tile_pool(name="psC", bufs=1, space="PSUM") as psC:

                ps_m = psB.tile([128, NM], F32, name="ps_m")
                ps_tc = psC.tile([128, 2], F32, name="ps_tc")
                ps_t = ps_tc[:, 0:1]
                ps_corr = ps_tc[:, 1:2]
                ps_row = ps_m[0:1, 0:512]

                # slot-a pair exp rides the warm EXP table before the swap
                pea = sml.tile([128, NBLK], BF16, name="pea")
                nc.scalar.activation(pea[:], pair2[:], AF.Exp,
                                     bias=zero, scale=ga)
                # zb: zero bias that depends on the last EXP-block op, so
                # the scheduler cannot move the SQRT block earlier
                zb1 = sml.tile([128, 1], F32, name="zb1")
                nc.vector.tensor_scalar(
                    out=zb1[:], in0=kta[:, 6 * NM - 1:6 * NM],
                    scalar1=0.0, scalar2=0.0, op0=ALU.mult, op1=ALU.add)
                zb = sml.tile([128, 1], F32, name="zb")
                nc.vector.tensor_tensor(out=zb[:], in0=zb1[:],
                                        in1=pea[:, 0:1], op=ALU.mult)

                # slot-a M0 runs after the whole d2 phase (in the idle PE
                # sqrt window): interleaved, each d2 matmul would chain
                # behind the previous exp in the PE's in-order queue. The
                # zb-gated copy of atp pins the ordering.
                atp2 = scr.tile([128, 6 * D], BF16, name="atp2")
                nc.vector.tensor_scalar(
                    out=atp2[:], in0=atp[:], scalar1=1.0, scalar2=zb[:],
                    op0=ALU.mult, op1=ALU.add)
                for r in range(6):
                    for fs in (slice(0, 512), slice(512, NM)):
                        nc.tensor.matmul(ps_m[0:64, fs],
                                         atp2[:, D * r:D * r + 64],
                                         kta[:, NM * r + fs.start:
                                              NM * r + fs.stop],
                                         start=(r == 0), stop=(r == 5),
                                         tile_position=(0, 0),
                                         skip_group_check=True)

                # ---- swap to SQRT: dist halves, then pair dist ----
                for h in range(2):
                    hs = slice(3 * NM * h, 3 * NM * (h + 1))
                    nc.scalar.activation(dist[:, hs], d2sb[:, hs], AF.Sqrt,
                                         bias=zb, scale=1.0)
                zb2 = sml.tile([128, 1], F32, name="zb2")
                nc.vector.tensor_scalar(
                    out=zb2[:], in0=dist[:, 6 * NM - 1:6 * NM],
                    scalar1=0.0, scalar2=0.0, op0=ALU.mult, op1=ALU.add)
                pdist = sml.tile([128, NBLK], F32, name="pdist")
                nc.scalar.activation(pdist[:], pair2[:], AF.Sqrt,
                                     bias=zb2, scale=1.0)
                # lbt == lb, but depends on the last SQRT-block op
                lbt = sml.tile([128, 1], F32, name="lbt")
                nc.vector.tensor_scalar(
                    out=lbt[:], in0=pdist[:, 0:1], scalar1=0.0,
                    scalar2=lb, op0=ALU.mult, op1=ALU.add)

                def slot_tail(i, pe):
                    """Pair sums, corrections, row stats, ubv and the PE
                    transpose for slot i; runs as soon as its M0 stops."""
                    pt = slice(64 * i, 64 * i + 64)
                    # t3: per-perm 3-block partial sums; group PPC holds the
                    # stripe so t[50] = sum(e) lands in ps_t for free
                    pe3 = pe.rearrange("p (g t) -> p g t", t=3)
                    t3 = sml.tile([128, 64], BF16, name=f"t3_{i}")
                    nc.vector.memset(t3[:, PPC + 1:64], 0.0)
                    with nc.allow_low_precision(reason="3-wide bf16 sum"):
                        nc.vector.tensor_reduce(t3[:, 0:PPC + 1], pe3[:],
                                                axis=mybir.AxisListType.X,
                                                op=ALU.add)
                    nc.tensor.matmul(ps_t[pt, :], t3[:], onesb[:],
                                     start=True, stop=True,
                                     tile_position=(0, 64 * i),
                                     skip_group_check=True)
                    for c in range(3):
                        nc.tensor.matmul(
                            ps_corr[pt, :], wct[:, D * c:D * c + 64],
                            pe[:, 3 * PPC + c:3 * PPC + c + 1],
                            start=(c == 0), stop=(c == 2),
                            tile_position=(0, 64 * i),
                            skip_group_check=True)
                    # row stats off this slot's half of ps_m; slot b uses
                    # the (idle) Scalar engine for the copy+rowsum so only
                    # sA rides the DVE tail; q0 reads PSUM directly
                    if i == 0:
                        nc.vector.tensor_scalar(
                            out=M0sb[pt, :], in0=ps_m[pt, :], scalar1=1.0,
                            scalar2=0.0, op0=ALU.mult, op1=ALU.add,
                            accum_out=arow[pt, :])
                    else:
                        nc.scalar.activation(M0sb[pt, :], ps_m[pt, :],
                                             AF.Identity, bias=zero[pt],
                                             scale=1.0,
                                             accum_out=arow[pt, :])
                    nc.vector.scalar_tensor_tensor(
                        out=sB[pt, :], in0=ps_m[pt, :], scalar=1.0,
                        in1=astk[pt, :], op0=ALU.mult, op1=ALU.mult,
                        accum_out=q0c[pt, :])
                    nc.vector.tensor_scalar(
                        out=sA[pt, :], in0=M0sb[pt, 0:N], scalar1=1.0,
                        scalar2=0.0, op0=ALU.mult, op1=ALU.add,
                        accum_out=colA[pt, :])
                    # ubv = KAP*(q0 - arow) + corr + TCO*t into pack col 0;
                    # colA / colB = arow - colA / t into cols 1-3
                    nc.vector.tensor_tensor(out=pack[pt, 0:1], in0=q0c[pt, :],
                                            in1=arow[pt, :], op=ALU.subtract)
                    nc.vector.scalar_tensor_tensor(
                        out=pack[pt, 0:1], in0=pack[pt, 0:1],
                        scalar=float(KAP), in1=ps_corr[pt, :],
                        op0=ALU.mult, op1=ALU.add)
                    nc.vector.scalar_tensor_tensor(
                        out=pack[pt, 0:1], in0=ps_t[pt, :],
                        scalar=float(TCO), in1=pack[pt, 0:1],
                        op0=ALU.mult, op1=ALU.add)
                    nc.vector.tensor_copy(pack[pt, 3:4], ps_t[pt, :])
                    nc.vector.tensor_tensor(out=pack[pt, 2:3],
                                            in0=arow[pt, :],
                                            in1=colA[pt, :], op=ALU.subtract)
                    nc.vector.tensor_copy(pack[pt, 1:2], colA[pt, :])
                    # transpose the 4 pack columns into the partition-0 row
                    # (ps_m bank 0 is free again: stats above read it first)
                    for k in range(4):
                        nc.tensor.matmul(
                            ps_row[0:1,
                                   128 * k + 64 * i:128 * k + 64 * i + 64],
                            pack[pt, k:k + 1], idm[pt, :],
                            is_transpose=True, start=True, stop=True,
                            tile_position=(64 * i, 0),
                            skip_group_check=True)

                slot_tail(0, pea)

                # ---- swap back to EXP: slot-b K chunks + pair exp ----
                for r in range(6):
                    sl = slice(NM * r, NM * (r + 1))
                    nc.scalar.activation(ktb[:, sl], dist[:, sl], AF.Exp,
                                         scale=lbt, bias=zero)
                    for fs in (slice(0, 512), slice(512, NM)):
                        nc.tensor.matmul(ps_m[64:128, fs],
                                         atp[:, D * r:D * r + 64],
                                         ktb[:, NM * r + fs.start:
                                              NM * r + fs.stop],
                                         start=(r == 0), stop=(r == 5),
                                         tile_position=(0, 64),
                                         skip_group_check=True)
                peb = sml.tile([128, NBLK], BF16, name="peb")
                nc.scalar.activation(peb[:], pdist[:], AF.Exp,
                                     bias=zero, scale=lbt)

                slot_tail(1, peb)

                # ---- partition-0 assembly ----
                frow = sml.tile([1, 512], F32, name="frow")
                nc.vector.tensor_copy(frow[:], ps_row)

                def strided(row, col, *rest):
                    ap = frow[0:1, 128 * row + col:128 * row + col + 1]
                    return bass.AP(ap.tensor, ap.offset,
                                   [ap.ap[0], *rest])

                XXv = strided(1, PPC, [64, 2])
                XY0v = strided(2, PPC, [64, 2])
                YYv = strided(2, PPC + 1, [64, 2])
                sev = strided(3, PPC, [64, 2])
                # s0t = XX + YX + XY0 + YY in one grouped reduce
                quad = strided(1, PPC, [64, 2], [128, 2], [1, 2])
                s0t = sml.tile([1, 2], F32, name="s0t")
                nc.vector.tensor_reduce(s0t[:], quad,
                                        axis=mybir.AxisListType.XY,
                                        op=ALU.add)
                ck = sml.tile([1, 2], F32, name="ck")
                nc.vector.tensor_tensor(out=ck[:], in0=s0t[:], in1=sev,
                                        op=ALU.subtract)
                nc.vector.scalar_tensor_tensor(
                    out=ck[:], in0=ck[:], scalar=float(IC1), in1=aux4i,
                    op0=ALU.mult, op1=ALU.subtract)
                u1 = sml.tile([1, 2], F32, name="u1")
                nc.vector.tensor_tensor(out=u1[:], in0=XXv, in1=YYv,
                                        op=ALU.add)
                nc.vector.scalar_tensor_tensor(
                    out=u1[:], in0=u1[:], scalar=float(IC1), in1=aux4i,
                    op0=ALU.mult, op1=ALU.subtract)
                u2 = sml.tile([1, 2], F32, name="u2")
                nc.vector.tensor_tensor(out=u2[:], in0=XY0v, in1=sev,
                                        op=ALU.subtract)
                uF = sml.tile([1, 2], F32, name="uF")
                nc.vector.scalar_tensor_tensor(
                    out=uF[:], in0=u2[:], scalar=float(-2.0 * IC2), in1=u1[:],
                    op0=ALU.mult, op1=ALU.add)

                ubc = sml.tile([1, 2 * PPC], F32, name="ubc")
                ub0 = frow[0:1, 0:1]
                ub_src = bass.AP(ub0.tensor, ub0.offset,
                                 [ub0.ap[0], [64, 2], [1, PPC]])
                ckap = ck[0:1, 0:2]
                ck_b = bass.AP(ckap.tensor, ckap.offset,
                               [ckap.ap[0], [1, 2], [0, PPC]])
                nc.vector.tensor_tensor(
                    out=ubc[0:1, :].rearrange("o (k p) -> o k p", p=PPC),
                    in0=ub_src, in1=ck_b, op=ALU.add)
                nc.sync.dma_start(
                    out=out_d[:, 0:1],
                    in_=uF[0:1, :].rearrange("o (k w) -> o k w", w=1))
                nc.sync.dma_start(
                    out=out_d[:, 1:1 + PPC],
                    in_=ubc[0:1, :].rearrange("o (k p) -> o k p", p=PPC))

    nc.compile()
    return nc


def _host_prep(X, Y, bandwidths, perms):
    X = np.ascontiguousarray(X, np.float32)
    Y = np.ascontiguousarray(Y, np.float32)
    perms = np.ascontiguousarray(perms, np.int32)
    Zf = np.concatenate([X, Y], 0)
    Zt = Zf.T.astype(np.float32)
    sq = (Zf.astype(np.float64) ** 2).sum(1).astype(np.float32)
    b = np.asarray(bandwidths, np.float64)

    zlr = np.zeros((D + 1, 2 * NM), np.float32)
    zlr[0:D, NM:] = Zt
    zlr[D, NM:] = 1.0
    zlr[0:D, 0:NM] = -2.0 * Zt
    zlr[D, 0:NM] = sq + BIAS

    idm = np.tile(np.eye(64, dtype=np.float32), (2, 1))

    maps = []
    for cid in range(NC):
        ka, kb = (0, 1) if cid < 4 else (2, 3)
        q = cid % 4
        pm = perms[q * PPC:(q + 1) * PPC]

        A = np.zeros((ROWS, NM), np.float32)
        A[np.arange(PPC)[:, None], pm[:, :N]] = 1
        A[PPC, :N] = 1
        A[PPC + 1, N:] = 1
        astk = np.zeros((128, NM), np.float32)
        astk[0:ROWS] = A
        astk[64:64 + ROWS] = A
        atp = np.zeros((128, 6 * D), np.float32)
        for c in range(6):
            atp[:, D * c:D * c + ROWS] = A[:, 128 * c:128 * (c + 1)].T
        A1 = A[:PPC, :N]
        A2 = A[:PPC, N:]
        Wc = (-KAP * (A1 * A2) + CB1 * A1 + CB2 * A2).astype(np.float32)
        wct = np.zeros((128, 3 * D), np.float32)
        for c in range(3):
            wct[:, D * c:D * c + PPC] = Wc[:, 128 * c:128 * (c + 1)].T
        bfp = np.zeros((128, NM + 6 * D + 3 * D), np.float32)
        bfp[:, 0:NM] = astk
        bfp[:, NM:NM + 6 * D] = atp
        bfp[:, NM + 6 * D:NM + 9 * D] = wct

        # pair partial squares: perm p pair j at lane (384p+j)%128, block
        # (384p+j)//128, 4 groups of 16 dims. Stripe pairs (j, 384+j) fill
        # blocks 3*PPC..3*PPC+2; stripe hits inside perm rows get a huge
        # sentinel so exp -> 0 (the zeroed K stripe).
        pX = pm[:, :N].astype(np.int64).ravel()
        pY = pm[:, N:].astype(np.int64).ravel()
        pdv = (Zf[pX] - Zf[pY]).astype(np.float32) ** 2
        psq = pdv.reshape(-1, 4, 16).sum(2) + np.float32(BIAS / 4)
        psq[pY == pX + N] = 1e6
        sdv = (Zf[:N] - Zf[N:]).astype(np.float32) ** 2
        psq = np.concatenate(
            [psq, sdv.reshape(-1, 4, 16).sum(2) + np.float32(BIAS / 4)], 0)
        psq = psq.reshape(NBLK, 128, 4).transpose(1, 0, 2).reshape(128, -1)

        fsp = np.zeros((128, 32), np.float32)
        ga = np.float32(-1.0 / (b[ka] * b[ka]))
        lb = np.float32(-1.0 / b[kb])
        sqcols = sq.reshape(6, 128).T
        fsp[:, 0:6] = sqcols
        fsp[:, 6:12] = ga * sqcols
        fsp[:, 12] = ga
        fsp[:, 13] = lb
        fsp[:, 14] = 0.0
        d0a = np.exp(-BIAS / (b[ka] * b[ka]))
        d0b = np.exp(-np.sqrt(BIAS) / b[kb])
        fsp[0, 16] = np.float32(NM * d0a * IC1)
        fsp[0, 17] = np.float32(NM * d0b * IC1)

        maps.append(dict(zlr=zlr, psq=psq.astype(ml_dtypes.bfloat16),
                         bfp=bfp.astype(ml_dtypes.bfloat16), fsp=fsp,
                         idm=idm))
    return maps


_NC_CACHE = None


def _get_nc():
    global _NC_CACHE
    if _NC_CACHE is None:
        _NC_CACHE = _build()
    return _NC_CACHE


def _merge(results):
    full = np.zeros((4, 1 + NPER), np.float32)
    for cid in range(NC):
        ka, kb = (0, 1) if cid < 4 else (2, 3)
        q = cid % 4
        o = results[cid]["out"]
        full[ka, 1 + q * PPC:1 + (q + 1) * PPC] = o[0, 1:]
        full[kb, 1 + q * PPC:1 + (q + 1) * PPC] = o[1, 1:]
        if q == 0:
            full[ka, 0] = o[0, 0]
            full[kb, 0] = o[1, 0]
    return full


def kernel(X, Y, bandwidths, perms):
    nc = _get_nc()
    in_maps = _host_prep(X, Y, bandwidths, perms)
    res = bass_utils.run_bass_kernel_spmd(nc, in_maps, list(range(NC)))
    return _merge(res.results)


# revision 27
# speedup vs baseline: 1.1563x; 1.1563x over previous
"""Trainium2 Bass kernel for the 4-kernel MMD permutation test (nn_DUAL_78237124264373).

Sharding: 8 cores = 2 kernel-pairs x 4 permutation quarters. Core c<4 computes
kernels (0,1) [gaussian, laplacian] for perms [50*(c%4), 50*(c%4)+50); core
c>=4 the same for kernels (2,3). The host merges the [2, 1+50] per-core
outputs, so each core only ever evaluates TWO kernel matrices and the
activation-table sequence is exactly EXP (pre-warmed) -> SQRT -> EXP: the
swap points are pinned with zero-valued bias/scale tokens that data-depend
on the previous block's last op, so the Tile scheduler cannot interleave.

Per-core pipeline (slot a = gaussian, slot b = laplacian):
  d2 = L^T R on the PE in f32r (L = [Zt; 1], R = [-2 Zt; sq + B]), 12 PSUM
  pieces (6 row tiles x 512+256) in a 4-deep PSUM pool. Slot-a K = exp(ga*d2)
  straight out of PSUM with its M0 = A_aug K chunk matmul right behind, while
  the DVE lands clamped d2 in SBUF for the sqrt block. After the swap back to
  EXP, slot-b K = exp(lb*dist) runs chunk-by-chunk with M0 trailing. Each
  slot's row stats (aKa, aK1, colA), U_b vector and PE transpose into the
  partition-0 assembly row run as soon as that slot's M0 completes, so slot
  a's tail hides under the SQRT/slot-b window. U_b = KAP*(aKa - aK1) +
  W_corr @ e + (2/c2)*t + ck, with pair sums t reduced from host-gathered
  16-wide partial squares and e the K0[j, 384+j] stripe (3 extra pair
  blocks). The final scalar assembly (U, ck) happens on partition 0.
"""

import sys

import numpy as np

if "/opt/trn_rl_repo" not in sys.path:
    sys.path.insert(0, "/opt/trn_rl_repo")

import ml_dtypes

import concourse.bacc as bacc
import concourse.bass as bass
import concourse.mybir as mybir
import concourse.tile as tile
from concourse import bass_utils

N = 384
NM = 768
D = 64
NPER = 200
NC = 8
PPC = 50                      # perms per core
ROWS = PPC + 2                # + X-identity + Y-identity rows
NBLK = 3 * PPC + 3            # pair blocks of 128: 50 perms x 3 + stripe x 3
BIAS = 1e-3                   # keeps d2 > 0 under f32r rounding (see d0c)
C1 = float(N * (N - 1))
C2 = float(N * N)
KAP = np.float32(2.0 / C1 + 2.0 / C2)
CB1 = np.float32(1.0 / C1 + 2.0 / C2)
CB2 = np.float32(1.0 / C1)
TCO = np.float32(2.0 / C2)
IC1 = np.float32(1.0 / C1)
IC2 = np.float32(1.0 / C2)

F32 = mybir.dt.float32
F32R = mybir.dt.float32r
BF16 = mybir.dt.bfloat16
AF = mybir.ActivationFunctionType
ALU = mybir.AluOpType


def _build():
    nc = bacc.Bacc("TRN2", target_bir_lowering=False, debug=False)
    with tile.TileContext(nc) as tc:
        with tc.tile_pool(name="dram", bufs=1, space="DRAM") as dram, \
             tc.tile_pool(name="io", bufs=1) as io, \
             tc.tile_pool(name="big", bufs=1) as big, \
             tc.tile_pool(name="scr", bufs=1) as scr, \
             tc.tile_pool(name="sml", bufs=1) as sml:

            def din(name, shape, dt=F32):
                return dram.tile(shape, dt, kind="ExternalInput", name=name,
                                 uniquify=False)

            zlr_d = din("zlr", [D + 1, 2 * NM], F32R)
            psq_d = din("psq", [128, NBLK * 4], BF16)
            bfp_d = din("bfp", [128, NM + 6 * D + 3 * D], BF16)
            fsp_d = din("fsp", [128, 32], F32)
            idm_d = din("idm", [128, 64], F32)
            out_d = dram.tile([2, 1 + PPC], F32, kind="ExternalOutput",
                              name="out", uniquify=False)

            # ---- input DMAs; zlr is [R | L] so the first piece lands first
            fsp = io.tile([128, 32], F32, name="fsp_sb")
            nc.sync.dma_start(out=fsp[:], in_=fsp_d[:])
            zlr = io.tile([D + 1, 2 * NM], F32R, name="zlr_sb")
            nc.sync.dma_start(out=zlr[:, 0:896], in_=zlr_d[:, 0:896])
            nc.sync.dma_start(out=zlr[:, 896:], in_=zlr_d[:, 896:])
            psq = io.tile([128, NBLK * 4], BF16, name="psq_sb")
            nc.sync.dma_start(out=psq[:], in_=psq_d[:])
            bfp = io.tile([128, NM + 6 * D + 3 * D], BF16, name="bfp_sb")
            nc.sync.dma_start(out=bfp[:], in_=bfp_d[:])
            idm = io.tile([128, 64], F32, name="idm_sb")
            nc.sync.dma_start(out=idm[:], in_=idm_d[:])

            zr = zlr[:, 0:NM]
            zl = zlr[:, NM:2 * NM]
            astk = bfp[:, 0:NM]                      # A rows at 0-51 / 64-115
            atp = bfp[:, NM:NM + 6 * D]              # A^T chunks, 64-padded
            wct = bfp[:, NM + 6 * D:NM + 9 * D]      # W_corr^T chunks
            sqc = fsp[:, 0:6]                        # sq columns per row tile
            gbias = fsp[:, 6:12]                     # ga * sq per row tile
            ga = fsp[:, 12:13]
            lb = fsp[:, 13:14]
            zero = fsp[:, 14:15]
            aux4i = fsp[0:1, 16:18]                  # 768*d0c*IC1 per slot

            ones = io.tile([128, 1], F32, name="ones_sb")
            nc.vector.memset(ones[:], 1.0)
            onesb = io.tile([128, 1], BF16, name="onesb_sb")
            nc.vector.memset(onesb[:], 1.0)

            d2sb = big.tile([128, 6 * NM], F32, name="d2sb")
            dist = big.tile([128, 6 * NM], F32, name="dist_sb")
            kta = big.tile([128, 6 * NM], BF16, name="kta")
            ktb = big.tile([128, 6 * NM], BF16, name="ktb")
            M0sb = big.tile([128, NM], F32, name="M0sb")
            sA = scr.tile([128, N], F32, name="sA")
            sB = scr.tile([128, NM], F32, name="sB")
            pair2 = sml.tile([128, NBLK], F32, name="pair2")
            arow = sml.tile([128, 1], F32, name="arow")
            colA = sml.tile([128, 1], F32, name="colA")
            q0c = sml.tile([128, 1], F32, name="q0c")
            pack = sml.tile([128, 4], F32, name="pack")
            frow = sml.tile([1, 512], F32, name="frow")

            # warm the EXP activation table while DMAs are in flight
            warm = sml.tile([128, 1], F32, name="warm")
            nc.scalar.activation(warm[0:1, :], ones[0:1, :], AF.Exp,
                                 bias=0.0, scale=1.0)

            # ---- pair d2: reduce the host 16-wide partial squares ----
            psq3 = psq.rearrange("p (b d) -> p b d", d=4)
            nc.vector.tensor_reduce(pair2[:], psq3[:],
                                    axis=mybir.AxisListType.X, op=ALU.add)

            # ---- d2 phase: f32r matmuls, one [128,768] PSUM tile per row
            # tile, 3 deep; slot-a exp from PSUM; DVE lands d2 for sqrt ----
            with tc.tile_pool(name="psA", bufs=3, space="PSUM") as psA:
                for r in range(6):
                    lhs = zl[:, 128 * r:128 * (r + 1)]
                    ps_d2 = psA.tile([128, NM], F32, tag="d2",
                                     name=f"ps_d2_{r}")
                    nc.tensor.matmul(ps_d2[:, 0:512], lhs, zr[:, 0:512],
                                     start=True, stop=True)
                    nc.tensor.matmul(ps_d2[:, 512:NM], lhs, zr[:, 512:NM],
                                     start=True, stop=True)
                    sl = slice(NM * r, NM * (r + 1))
                    nc.scalar.activation(kta[:, sl], ps_d2[:], AF.Exp,
                                         scale=ga, bias=gbias[:, r:r + 1])
                    nc.vector.tensor_scalar(
                        out=d2sb[:, sl], in0=ps_d2[:],
                        scalar1=sqc[:, r:r + 1], scalar2=0.0,
                        op0=ALU.add, op1=ALU.max)


            with tc.tile_pool(name="psB", bufs=1, space="PSUM") as psB, \
                 tc.tile_pool(name="psC", bufs=1, space="PSUM") as psC:

                ps_m = psB.tile([128, NM], F32, name="ps_m")
                ps_tc = psC.tile([128, 2], F32, name="ps_tc")
                ps_t = ps_tc[:, 0:1]
                ps_corr = ps_tc[:, 1:2]
                ps_row = ps_m[0:1, 0:512]

                # slot-a pair exp rides the warm EXP table before the swap
                pea = sml.tile([128, NBLK], BF16, name="pea")
                nc.scalar.activation(pea[:], pair2[:], AF.Exp,
                                     bias=zero, scale=ga)
                # zb: zero bias that depends on the last EXP-block op, so
                # the scheduler cannot move the SQRT block earlier
                zb1 = sml.tile([128, 1], F32, name="zb1")
                nc.gpsimd.tensor_scalar(
                    out=zb1[:], in0=kta[:, 6 * NM - 1:6 * NM],
                    scalar1=0.0, scalar2=0.0, op0=ALU.mult, op1=ALU.add)
                zb = sml.tile([128, 1], F32, name="zb")
                nc.gpsimd.tensor_tensor(out=zb[:], in0=zb1[:],
                                        in1=pea[:, 0:1], op=ALU.mult)

                # slot-a M0 runs after the whole d2 phase (in the idle PE
                # sqrt window): interleaved, each d2 matmul would chain
                # behind the previous exp in the PE's in-order queue. The
                # zb-gated copy of atp pins the ordering.
                atp2 = scr.tile([128, 6 * D], BF16, name="atp2")
                nc.gpsimd.tensor_scalar(
                    out=atp2[:], in0=atp[:], scalar1=1.0, scalar2=zb[:],
                    op0=ALU.mult, op1=ALU.add)
                for r in range(6):
                    for fs in (slice(0, 512), slice(512, NM)):
                        nc.tensor.matmul(ps_m[0:64, fs],
                                         atp2[:, D * r:D * r + 64],
                                         kta[:, NM * r + fs.start:
                                              NM * r + fs.stop],
                                         start=(r == 0), stop=(r == 5),
                                         tile_position=(0, 0),
                                         skip_group_check=True)

                # ---- swap to SQRT: dist halves, then pair dist ----
                for h in range(2):
                    hs = slice(3 * NM * h, 3 * NM * (h + 1))
                    nc.scalar.activation(dist[:, hs], d2sb[:, hs], AF.Sqrt,
                                         bias=zb, scale=1.0)
                zb2 = sml.tile([128, 1], F32, name="zb2")
                nc.gpsimd.tensor_scalar(
                    out=zb2[:], in0=dist[:, 6 * NM - 1:6 * NM],
                    scalar1=0.0, scalar2=0.0, op0=ALU.mult, op1=ALU.add)
                pdist = sml.tile([128, NBLK], F32, name="pdist")
                nc.scalar.activation(pdist[:], pair2[:], AF.Sqrt,
                                     bias=zb2, scale=1.0)
                # lbt == lb, but depends on the last SQRT-block op
                lbt = sml.tile([128, 1], F32, name="lbt")
                nc.gpsimd.tensor_scalar(
                    out=lbt[:], in0=pdist[:, 0:1], scalar1=0.0,
                    scalar2=lb, op0=ALU.mult, op1=ALU.add)

                def slot_tail(i, pe):
                    """Pair sums, corrections, row stats, ubv and the PE
                    transpose for slot i; runs as soon as its M0 stops."""
                    pt = slice(64 * i, 64 * i + 64)
                    # t3: per-perm 3-block partial sums; group PPC holds the
                    # stripe so t[50] = sum(e) lands in ps_t for free
                    pe3 = pe.rearrange("p (g t) -> p g t", t=3)
                    t3 = sml.tile([128, 64], BF16, name=f"t3_{i}")
                    nc.vector.memset(t3[:, PPC + 1:64], 0.0)
                    with nc.allow_low_precision(reason="3-wide bf16 sum"):
                        nc.vector.tensor_reduce(t3[:, 0:PPC + 1], pe3[:],
                                                axis=mybir.AxisListType.X,
                                                op=ALU.add)
                    nc.tensor.matmul(ps_t[pt, :], t3[:], onesb[:],
                                     start=True, stop=True,
                                     tile_position=(0, 64 * i),
                                     skip_group_check=True)
                    for c in range(3):
                        nc.tensor.matmul(
                            ps_corr[pt, :], wct[:, D * c:D * c + 64],
                            pe[:, 3 * PPC + c:3 * PPC + c + 1],
                            start=(c == 0), stop=(c == 2),
                            tile_position=(0, 64 * i),
                            skip_group_check=True)
                    # row stats off this slot's half of ps_m; slot b uses
                    # the (idle) Scalar engine for the copy+rowsum so only
                    # sA rides the DVE tail; q0 reads PSUM directly
                    nc.vector.scalar_tensor_tensor(
                        out=sB[pt, :], in0=ps_m[pt, :], scalar=1.0,
                        in1=astk[pt, :], op0=ALU.mult, op1=ALU.mult,
                        accum_out=q0c[pt, :])
                    if i == 0:
                        nc.vector.tensor_scalar(
                            out=M0sb[pt, :], in0=ps_m[pt, :], scalar1=1.0,
                            scalar2=0.0, op0=ALU.mult, op1=ALU.add,
                            accum_out=arow[pt, :])
                    else:
                        nc.scalar.activation(M0sb[pt, :], ps_m[pt, :],
                                             AF.Identity, bias=zero[pt],
                                             scale=1.0,
                                             accum_out=arow[pt, :])
                    nc.vector.tensor_scalar(
                        out=sA[pt, :], in0=ps_m[pt, 0:N], scalar1=1.0,
                        scalar2=0.0, op0=ALU.mult, op1=ALU.add,
                        accum_out=colA[pt, :])
                    # ubv = KAP*(q0 - arow) + corr + TCO*t into pack col 0;
                    # colA / colB = arow - colA / t into cols 1-3
                    nc.vector.tensor_tensor(out=pack[pt, 0:1], in0=q0c[pt, :],
                                            in1=arow[pt, :], op=ALU.subtract)
                    nc.vector.scalar_tensor_tensor(
                        out=pack[pt, 0:1], in0=pack[pt, 0:1],
                        scalar=float(KAP), in1=ps_corr[pt, :],
                        op0=ALU.mult, op1=ALU.add)
                    nc.vector.scalar_tensor_tensor(
                        out=pack[pt, 0:1], in0=ps_t[pt, :],
                        scalar=float(TCO), in1=pack[pt, 0:1],
                        op0=ALU.mult, op1=ALU.add)
                    nc.vector.tensor_copy(pack[pt, 3:4], ps_t[pt, :])
                    nc.vector.tensor_tensor(out=pack[pt, 2:3],
                                            in0=arow[pt, :],
                                            in1=colA[pt, :], op=ALU.subtract)
                    nc.vector.tensor_copy(pack[pt, 1:2], colA[pt, :])
                    # transpose the 4 pack columns into the partition-0 row
                    # (ps_m bank 0 is free again: stats above read it first)
                    for k in range(4):
                        nc.tensor.matmul(
                            ps_row[0:1,
                                   128 * k + 64 * i:128 * k + 64 * i + 64],
                            pack[pt, k:k + 1], idm[pt, :],
                            is_transpose=True, start=True, stop=True,
                            tile_position=(64 * i, 0),
                            skip_group_check=True)
                    s0 = ps_row[0:1, 64 * i:64 * i + 1]
                    f0 = frow[0:1, 64 * i:64 * i + 1]
                    nc.vector.tensor_copy(
                        bass.AP(f0.tensor, f0.offset,
                                [f0.ap[0], [128, 4], [1, 64]]),
                        bass.AP(s0.tensor, s0.offset,
                                [s0.ap[0], [128, 4], [1, 64]]))

                slot_tail(0, pea)

                # ---- swap back to EXP: slot-b K chunks + pair exp ----
                for r in range(6):
                    sl = slice(NM * r, NM * (r + 1))
                    nc.scalar.activation(ktb[:, sl], dist[:, sl], AF.Exp,
                                         scale=lbt, bias=zero)
                    for fs in (slice(0, 512), slice(512, NM)):
                        nc.tensor.matmul(ps_m[64:128, fs],
                                         atp[:, D * r:D * r + 64],
                                         ktb[:, NM * r + fs.start:
                                              NM * r + fs.stop],
                                         start=(r == 0), stop=(r == 5),
                                         tile_position=(0, 64),
                                         skip_group_check=True)
                peb = sml.tile([128, NBLK], BF16, name="peb")
                nc.scalar.activation(peb[:], pdist[:], AF.Exp,
                                     bias=zero, scale=lbt)

                slot_tail(1, peb)

                # ---- partition-0 assembly ----
                def strided(row, col, *rest):
                    ap = frow[0:1, 128 * row + col:128 * row + col + 1]
                    return bass.AP(ap.tensor, ap.offset,
                                   [ap.ap[0], *rest])

                XXv = strided(1, PPC, [64, 2])
                XY0v = strided(2, PPC, [64, 2])
                YYv = strided(2, PPC + 1, [64, 2])
                sev = strided(3, PPC, [64, 2])
                # s0t = XX + YX + XY0 + YY in one grouped reduce
                quad = strided(1, PPC, [64, 2], [128, 2], [1, 2])
                s0t = sml.tile([1, 2], F32, name="s0t")
                nc.vector.tensor_reduce(s0t[:], quad,
                                        axis=mybir.AxisListType.XY,
                                        op=ALU.add)
                ck = sml.tile([1, 2], F32, name="ck")
                nc.vector.tensor_tensor(out=ck[:], in0=s0t[:], in1=sev,
                                        op=ALU.subtract)
                nc.vector.scalar_tensor_tensor(
                    out=ck[:], in0=ck[:], scalar=float(IC1), in1=aux4i,
                    op0=ALU.mult, op1=ALU.subtract)
                u1 = sml.tile([1, 2], F32, name="u1")
                nc.vector.tensor_tensor(out=u1[:], in0=XXv, in1=YYv,
                                        op=ALU.add)
                nc.vector.scalar_tensor_tensor(
                    out=u1[:], in0=u1[:], scalar=float(IC1), in1=aux4i,
                    op0=ALU.mult, op1=ALU.subtract)
                u2 = sml.tile([1, 2], F32, name="u2")
                nc.vector.tensor_tensor(out=u2[:], in0=XY0v, in1=sev,
                                        op=ALU.subtract)
                uF = sml.tile([1, 2], F32, name="uF")
                nc.vector.scalar_tensor_tensor(
                    out=uF[:], in0=u2[:], scalar=float(-2.0 * IC2), in1=u1[:],
                    op0=ALU.mult, op1=ALU.add)

                ubc = sml.tile([1, 2 * PPC], F32, name="ubc")
                ub0 = frow[0:1, 0:1]
                ub_src = bass.AP(ub0.tensor, ub0.offset,
                                 [ub0.ap[0], [64, 2], [1, PPC]])
                ckap = ck[0:1, 0:2]
                ck_b = bass.AP(ckap.tensor, ckap.offset,
                               [ckap.ap[0], [1, 2], [0, PPC]])
                nc.vector.tensor_tensor(
                    out=ubc[0:1, :].rearrange("o (k p) -> o k p", p=PPC),
                    in0=ub_src, in1=ck_b, op=ALU.add)
                nc.sync.dma_start(
                    out=out_d[:, 0:1],
                    in_=uF[0:1, :].rearrange("o (k w) -> o k w", w=1))
                nc.sync.dma_start(
                    out=out_d[:, 1:1 + PPC],
                    in_=ubc[0:1, :].rearrange("o (k p) -> o k p", p=PPC))

    nc.compile()
    return nc


def _host_prep(X, Y, bandwidths, perms):
    X = np.ascontiguousarray(X, np.float32)
    Y = np.ascontiguousarray(Y, np.float32)
    perms = np.ascontiguousarray(perms, np.int32)
    Zf = np.concatenate([X, Y], 0)
    Zt = Zf.T.astype(np.float32)
    sq = (Zf.astype(np.float64) ** 2).sum(1).astype(np.float32)
    b = np.asarray(bandwidths, np.float64)

    zlr = np.zeros((D + 1, 2 * NM), np.float32)
    zlr[0:D, NM:] = Zt
    zlr[D, NM:] = 1.0
    zlr[0:D, 0:NM] = -2.0 * Zt
    zlr[D, 0:NM] = sq + BIAS

    idm = np.tile(np.eye(64, dtype=np.float32), (2, 1))

    maps = []
    for cid in range(NC):
        ka, kb = (0, 1) if cid < 4 else (2, 3)
        q = cid % 4
        pm = perms[q * PPC:(q + 1) * PPC]

        A = np.zeros((ROWS, NM), np.float32)
        A[np.arange(PPC)[:, None], pm[:, :N]] = 1
        A[PPC, :N] = 1
        A[PPC + 1, N:] = 1
        astk = np.zeros((128, NM), np.float32)
        astk[0:ROWS] = A
        astk[64:64 + ROWS] = A
        atp = np.zeros((128, 6 * D), np.float32)
        for c in range(6):
            atp[:, D * c:D * c + ROWS] = A[:, 128 * c:128 * (c + 1)].T
        A1 = A[:PPC, :N]
        A2 = A[:PPC, N:]
        Wc = (-KAP * (A1 * A2) + CB1 * A1 + CB2 * A2).astype(np.float32)
        wct = np.zeros((128, 3 * D), np.float32)
        for c in range(3):
            wct[:, D * c:D * c + PPC] = Wc[:, 128 * c:128 * (c + 1)].T
        bfp = np.zeros((128, NM + 6 * D + 3 * D), np.float32)
        bfp[:, 0:NM] = astk
        bfp[:, NM:NM + 6 * D] = atp
        bfp[:, NM + 6 * D:NM + 9 * D] = wct

        # pair partial squares: perm p pair j at lane (384p+j)%128, block
        # (384p+j)//128, 4 groups of 16 dims. Stripe pairs (j, 384+j) fill
        # blocks 3*PPC..3*PPC+2; stripe hits inside perm rows get a huge
        # sentinel so exp -> 0 (the zeroed K stripe).
        pX = pm[:, :N].astype(np.int64).ravel()
        pY = pm[:, N:].astype(np.int64).ravel()
        pdv = (Zf[pX] - Zf[pY]).astype(np.float32) ** 2
        psq = pdv.reshape(-1, 4, 16).sum(2) + np.float32(BIAS / 4)
        psq[pY == pX + N] = 1e6
        sdv = (Zf[:N] - Zf[N:]).astype(np.float32) ** 2
        psq = np.concatenate(
            [psq, sdv.reshape(-1, 4, 16).sum(2) + np.float32(BIAS / 4)], 0)
        psq = psq.reshape(NBLK, 128, 4).transpose(1, 0, 2).reshape(128, -1)

        fsp = np.zeros((128, 32), np.float32)
        ga = np.float32(-1.0 / (b[ka] * b[ka]))
        lb = np.float32(-1.0 / b[kb])
        sqcols = sq.reshape(6, 128).T
        fsp[:, 0:6] = sqcols
        fsp[:, 6:12] = ga * sqcols
        fsp[:, 12] = ga
        fsp[:, 13] = lb
        fsp[:, 14] = 0.0
        d0a = np.exp(-BIAS / (b[ka] * b[ka]))
        d0b = np.exp(-np.sqrt(BIAS) / b[kb])
        fsp[0, 16] = np.float32(NM * d0a * IC1)
        fsp[0, 17] = np.float32(NM * d0b * IC1)

        maps.append(dict(zlr=zlr, psq=psq.astype(ml_dtypes.bfloat16),
                         bfp=bfp.astype(ml_dtypes.bfloat16), fsp=fsp,
                         idm=idm))
    return maps


_NC_CACHE = None


def _get_nc():
    global _NC_CACHE
    if _NC_CACHE is None:
        _NC_CACHE = _build()
    return _NC_CACHE


def _merge(results):
    full = np.zeros((4, 1 + NPER), np.float32)
    for cid in range(NC):
        ka, kb = (0, 1) if cid < 4 else (2, 3)
        q = cid % 4
        o = results[cid]["out"]
        full[ka, 1 + q * PPC:1 + (q + 1) * PPC] = o[0, 1:]
        full[kb, 1 + q * PPC:1 + (q + 1) * PPC] = o[1, 1:]
        if q == 0:
            full[ka, 0] = o[0, 0]
            full[kb, 0] = o[1, 0]
    return full


def kernel(X, Y, bandwidths, perms):
    nc = _get_nc()
    in_maps = _host_prep(X, Y, bandwidths, perms)
    res = bass_utils.run_bass_kernel_spmd(nc, in_maps, list(range(NC)))
    return _merge(res.results)


# revision 28
# speedup vs baseline: 1.1565x; 1.0002x over previous
"""Trainium2 Bass kernel for the 4-kernel MMD permutation test (nn_DUAL_78237124264373).

Sharding: 8 cores = 2 kernel-pairs x 4 permutation quarters. Core c<4 computes
kernels (0,1) [gaussian, laplacian] for perms [50*(c%4), 50*(c%4)+50); core
c>=4 the same for kernels (2,3). The host merges the [2, 1+50] per-core
outputs, so each core only ever evaluates TWO kernel matrices and the
activation-table sequence is exactly EXP (pre-warmed) -> SQRT -> EXP: the
swap points are pinned with zero-valued bias/scale tokens that data-depend
on the previous block's last op, so the Tile scheduler cannot interleave.

Per-core pipeline (slot a = gaussian, slot b = laplacian):
  d2 = L^T R on the PE in f32r (L = [Zt; 1], R = [-2 Zt; sq + B]), 12 PSUM
  pieces (6 row tiles x 512+256) in a 4-deep PSUM pool. Slot-a K = exp(ga*d2)
  straight out of PSUM with its M0 = A_aug K chunk matmul right behind, while
  the DVE lands clamped d2 in SBUF for the sqrt block. After the swap back to
  EXP, slot-b K = exp(lb*dist) runs chunk-by-chunk with M0 trailing. Each
  slot's row stats (aKa, aK1, colA), U_b vector and PE transpose into the
  partition-0 assembly row run as soon as that slot's M0 completes, so slot
  a's tail hides under the SQRT/slot-b window. U_b = KAP*(aKa - aK1) +
  W_corr @ e + (2/c2)*t + ck, with pair sums t reduced from host-gathered
  16-wide partial squares and e the K0[j, 384+j] stripe (3 extra pair
  blocks). The final scalar assembly (U, ck) happens on partition 0.
"""

import sys

import numpy as np

if "/opt/trn_rl_repo" not in sys.path:
    sys.path.insert(0, "/opt/trn_rl_repo")

import ml_dtypes

import concourse.bacc as bacc
import concourse.bass as bass
import concourse.mybir as mybir
import concourse.tile as tile
from concourse import bass_utils

N = 384
NM = 768
D = 64
NPER = 200
NC = 8
PPC = 50                      # perms per core
ROWS = PPC + 2                # + X-identity + Y-identity rows
NBLK = 3 * PPC + 3            # pair blocks of 128: 50 perms x 3 + stripe x 3
BIAS = 1e-3                   # keeps d2 > 0 under f32r rounding (see d0c)
C1 = float(N * (N - 1))
C2 = float(N * N)
KAP = np.float32(2.0 / C1 + 2.0 / C2)
CB1 = np.float32(1.0 / C1 + 2.0 / C2)
CB2 = np.float32(1.0 / C1)
TCO = np.float32(2.0 / C2)
IC1 = np.float32(1.0 / C1)
IC2 = np.float32(1.0 / C2)

F32 = mybir.dt.float32
F32R = mybir.dt.float32r
BF16 = mybir.dt.bfloat16
AF = mybir.ActivationFunctionType
ALU = mybir.AluOpType


def _build():
    nc = bacc.Bacc("TRN2", target_bir_lowering=False, debug=False)
    with tile.TileContext(nc) as tc:
        with tc.tile_pool(name="dram", bufs=1, space="DRAM") as dram, \
             tc.tile_pool(name="io", bufs=1) as io, \
             tc.tile_pool(name="big", bufs=1) as big, \
             tc.tile_pool(name="scr", bufs=1) as scr, \
             tc.tile_pool(name="sml", bufs=1) as sml:

            def din(name, shape, dt=F32):
                return dram.tile(shape, dt, kind="ExternalInput", name=name,
                                 uniquify=False)

            zlr_d = din("zlr", [D + 1, 2 * NM], F32R)
            bfp_d = din("bfp", [128, NBLK * 4 + NM + 6 * D + 3 * D], BF16)
            fsp_d = din("fsp", [128, 32 + 64], F32)
            out_d = dram.tile([2, 1 + PPC], F32, kind="ExternalOutput",
                              name="out", uniquify=False)

            # ---- input DMAs; zlr is [R | L] so the first piece lands first
            fsp = io.tile([128, 32 + 64], F32, name="fsp_sb")
            nc.sync.dma_start(out=fsp[:], in_=fsp_d[:])
            zlr = io.tile([D + 1, 2 * NM], F32R, name="zlr_sb")
            nc.sync.dma_start(out=zlr[:, 0:896], in_=zlr_d[:, 0:896])
            nc.sync.dma_start(out=zlr[:, 896:], in_=zlr_d[:, 896:])
            bfp = io.tile([128, NBLK * 4 + NM + 6 * D + 3 * D], BF16,
                          name="bfp_sb")
            nc.sync.dma_start(out=bfp[:], in_=bfp_d[:])

            zr = zlr[:, 0:NM]
            zl = zlr[:, NM:2 * NM]
            idm = fsp[:, 32:96]
            psq = bfp[:, 0:NBLK * 4]
            PB = NBLK * 4
            astk = bfp[:, PB:PB + NM]                # A rows at 0-51 / 64-115
            atp = bfp[:, PB + NM:PB + NM + 6 * D]    # A^T chunks, 64-padded
            wct = bfp[:, PB + NM + 6 * D:PB + NM + 9 * D]
            sqc = fsp[:, 0:6]                        # sq columns per row tile
            gbias = fsp[:, 6:12]                     # ga * sq per row tile
            ga = fsp[:, 12:13]
            lb = fsp[:, 13:14]
            zero = fsp[:, 14:15]
            aux4i = fsp[0:1, 16:18]                  # 768*d0c*IC1 per slot

            ones = io.tile([128, 1], F32, name="ones_sb")
            nc.vector.memset(ones[:], 1.0)
            onesb = io.tile([128, 1], BF16, name="onesb_sb")
            nc.vector.memset(onesb[:], 1.0)

            d2sb = big.tile([128, 6 * NM], F32, name="d2sb")
            dist = big.tile([128, 6 * NM], F32, name="dist_sb")
            kta = big.tile([128, 6 * NM], BF16, name="kta")
            ktb = big.tile([128, 6 * NM], BF16, name="ktb")
            M0sb = big.tile([128, NM], F32, name="M0sb")
            sA = scr.tile([128, N], F32, name="sA")
            sB = scr.tile([128, NM], F32, name="sB")
            pair2 = sml.tile([128, NBLK], F32, name="pair2")
            arow = sml.tile([128, 1], F32, name="arow")
            colA = sml.tile([128, 1], F32, name="colA")
            q0c = sml.tile([128, 1], F32, name="q0c")
            pack = sml.tile([128, 4], F32, name="pack")
            frow = sml.tile([1, 512], F32, name="frow")

            # warm the EXP activation table while DMAs are in flight
            warm = sml.tile([128, 1], F32, name="warm")
            nc.scalar.activation(warm[0:1, :], ones[0:1, :], AF.Exp,
                                 bias=0.0, scale=1.0)

            # ---- pair d2: reduce the host 16-wide partial squares ----
            psq3 = psq.rearrange("p (b d) -> p b d", d=4)
            nc.vector.tensor_reduce(pair2[:], psq3[:],
                                    axis=mybir.AxisListType.X, op=ALU.add)

            # ---- d2 phase: f32r matmuls, one [128,768] PSUM tile per row
            # tile, 3 deep; slot-a exp from PSUM; DVE lands d2 for sqrt ----
            with tc.tile_pool(name="psA", bufs=3, space="PSUM") as psA:
                for r in range(6):
                    lhs = zl[:, 128 * r:128 * (r + 1)]
                    ps_d2 = psA.tile([128, NM], F32, tag="d2",
                                     name=f"ps_d2_{r}")
                    nc.tensor.matmul(ps_d2[:, 0:512], lhs, zr[:, 0:512],
                                     start=True, stop=True)
                    nc.tensor.matmul(ps_d2[:, 512:NM], lhs, zr[:, 512:NM],
                                     start=True, stop=True)
                    sl = slice(NM * r, NM * (r + 1))
                    nc.scalar.activation(kta[:, sl], ps_d2[:], AF.Exp,
                                         scale=ga, bias=gbias[:, r:r + 1])
                    nc.vector.tensor_scalar(
                        out=d2sb[:, sl], in0=ps_d2[:],
                        scalar1=sqc[:, r:r + 1], scalar2=0.0,
                        op0=ALU.add, op1=ALU.max)


            with tc.tile_pool(name="psB", bufs=1, space="PSUM") as psB, \
                 tc.tile_pool(name="psC", bufs=1, space="PSUM") as psC:

                ps_m = psB.tile([128, NM], F32, name="ps_m")
                ps_tc = psC.tile([128, 2], F32, name="ps_tc")
                ps_t = ps_tc[:, 0:1]
                ps_corr = ps_tc[:, 1:2]
                ps_row = ps_m[0:1, 0:512]

                # slot-a pair exp rides the warm EXP table before the swap
                pea = sml.tile([128, NBLK], BF16, name="pea")
                nc.scalar.activation(pea[:], pair2[:], AF.Exp,
                                     bias=zero, scale=ga)
                # zb: zero bias that depends on the last EXP-block op, so
                # the scheduler cannot move the SQRT block earlier
                zb1 = sml.tile([128, 1], F32, name="zb1")
                nc.gpsimd.tensor_scalar(
                    out=zb1[:], in0=kta[:, 6 * NM - 1:6 * NM],
                    scalar1=0.0, scalar2=0.0, op0=ALU.mult, op1=ALU.add)
                zb = sml.tile([128, 1], F32, name="zb")
                nc.gpsimd.tensor_tensor(out=zb[:], in0=zb1[:],
                                        in1=pea[:, 0:1], op=ALU.mult)

                # slot-a M0 runs after the whole d2 phase (in the idle PE
                # sqrt window): interleaved, each d2 matmul would chain
                # behind the previous exp in the PE's in-order queue. The
                # zb-gated copy of atp pins the ordering.
                atp2 = scr.tile([128, 6 * D], BF16, name="atp2")
                nc.gpsimd.tensor_scalar(
                    out=atp2[:], in0=atp[:], scalar1=1.0, scalar2=zb[:],
                    op0=ALU.mult, op1=ALU.add)
                for r in range(6):
                    for fs in (slice(0, 512), slice(512, NM)):
                        nc.tensor.matmul(ps_m[0:64, fs],
                                         atp2[:, D * r:D * r + 64],
                                         kta[:, NM * r + fs.start:
                                              NM * r + fs.stop],
                                         start=(r == 0), stop=(r == 5),
                                         tile_position=(0, 0),
                                         skip_group_check=True)

                # ---- swap to SQRT: dist halves, then pair dist ----
                for h in range(2):
                    hs = slice(3 * NM * h, 3 * NM * (h + 1))
                    nc.scalar.activation(dist[:, hs], d2sb[:, hs], AF.Sqrt,
                                         bias=zb, scale=1.0)
                zb2 = sml.tile([128, 1], F32, name="zb2")
                nc.gpsimd.tensor_scalar(
                    out=zb2[:], in0=dist[:, 6 * NM - 1:6 * NM],
                    scalar1=0.0, scalar2=0.0, op0=ALU.mult, op1=ALU.add)
                pdist = sml.tile([128, NBLK], F32, name="pdist")
                nc.scalar.activation(pdist[:], pair2[:], AF.Sqrt,
                                     bias=zb2, scale=1.0)
                # lbt == lb, but depends on the last SQRT-block op
                lbt = sml.tile([128, 1], F32, name="lbt")
                nc.gpsimd.tensor_scalar(
                    out=lbt[:], in0=pdist[:, 0:1], scalar1=0.0,
                    scalar2=lb, op0=ALU.mult, op1=ALU.add)

                def slot_tail(i, pe):
                    """Pair sums, corrections, row stats, ubv and the PE
                    transpose for slot i; runs as soon as its M0 stops."""
                    pt = slice(64 * i, 64 * i + 64)
                    # t3: per-perm 3-block partial sums; group PPC holds the
                    # stripe so t[50] = sum(e) lands in ps_t for free
                    pe3 = pe.rearrange("p (g t) -> p g t", t=3)
                    t3 = sml.tile([128, 64], BF16, name=f"t3_{i}")
                    nc.vector.memset(t3[:, PPC + 1:64], 0.0)
                    with nc.allow_low_precision(reason="3-wide bf16 sum"):
                        nc.vector.tensor_reduce(t3[:, 0:PPC + 1], pe3[:],
                                                axis=mybir.AxisListType.X,
                                                op=ALU.add)
                    nc.tensor.matmul(ps_t[pt, :], t3[:], onesb[:],
                                     start=True, stop=True,
                                     tile_position=(0, 64 * i),
                                     skip_group_check=True)
                    for c in range(3):
                        nc.tensor.matmul(
                            ps_corr[pt, :], wct[:, D * c:D * c + 64],
                            pe[:, 3 * PPC + c:3 * PPC + c + 1],
                            start=(c == 0), stop=(c == 2),
                            tile_position=(0, 64 * i),
                            skip_group_check=True)
                    # row stats off this slot's half of ps_m; slot b uses
                    # the (idle) Scalar engine for the copy+rowsum so only
                    # sA rides the DVE tail; q0 reads PSUM directly
                    nc.vector.scalar_tensor_tensor(
                        out=sB[pt, :], in0=ps_m[pt, :], scalar=1.0,
                        in1=astk[pt, :], op0=ALU.mult, op1=ALU.mult,
                        accum_out=q0c[pt, :])
                    nc.vector.tensor_scalar(
                        out=sA[pt, :], in0=ps_m[pt, 0:N], scalar1=1.0,
                        scalar2=0.0, op0=ALU.mult, op1=ALU.add,
                        accum_out=colA[pt, :])
                    if i == 0:
                        nc.vector.tensor_scalar(
                            out=M0sb[pt, :], in0=ps_m[pt, :], scalar1=1.0,
                            scalar2=0.0, op0=ALU.mult, op1=ALU.add,
                            accum_out=arow[pt, :])
                    else:
                        nc.scalar.activation(M0sb[pt, :], ps_m[pt, :],
                                             AF.Identity, bias=zero[pt],
                                             scale=1.0,
                                             accum_out=arow[pt, :])
                    # ubv = KAP*(q0 - arow) + corr + TCO*t into pack col 0;
                    # colA / colB = arow - colA / t into cols 1-3
                    nc.vector.tensor_tensor(out=pack[pt, 0:1], in0=q0c[pt, :],
                                            in1=arow[pt, :], op=ALU.subtract)
                    nc.vector.scalar_tensor_tensor(
                        out=pack[pt, 0:1], in0=pack[pt, 0:1],
                        scalar=float(KAP), in1=ps_corr[pt, :],
                        op0=ALU.mult, op1=ALU.add)
                    nc.vector.scalar_tensor_tensor(
                        out=pack[pt, 0:1], in0=ps_t[pt, :],
                        scalar=float(TCO), in1=pack[pt, 0:1],
                        op0=ALU.mult, op1=ALU.add)
                    nc.vector.tensor_copy(pack[pt, 3:4], ps_t[pt, :])
                    nc.vector.tensor_tensor(out=pack[pt, 2:3],
                                            in0=arow[pt, :],
                                            in1=colA[pt, :], op=ALU.subtract)
                    nc.vector.tensor_copy(pack[pt, 1:2], colA[pt, :])
                    # transpose the 4 pack columns into the partition-0 row
                    # (ps_m bank 0 is free again: stats above read it first)
                    for k in range(4):
                        nc.tensor.matmul(
                            ps_row[0:1,
                                   128 * k + 64 * i:128 * k + 64 * i + 64],
                            pack[pt, k:k + 1], idm[pt, :],
                            is_transpose=True, start=True, stop=True,
                            tile_position=(64 * i, 0),
                            skip_group_check=True)
                    s0 = ps_row[0:1, 64 * i:64 * i + 1]
                    f0 = frow[0:1, 64 * i:64 * i + 1]
                    nc.vector.tensor_copy(
                        bass.AP(f0.tensor, f0.offset,
                                [f0.ap[0], [128, 4], [1, 64]]),
                        bass.AP(s0.tensor, s0.offset,
                                [s0.ap[0], [128, 4], [1, 64]]))

                slot_tail(0, pea)

                # ---- swap back to EXP: slot-b K chunks + pair exp ----
                for c in range(3):
                    cs = slice(2 * NM * c, 2 * NM * (c + 1))
                    nc.scalar.activation(ktb[:, cs], dist[:, cs], AF.Exp,
                                         scale=lbt, bias=zero)
                    for r in (2 * c, 2 * c + 1):
                        for fs in (slice(0, 512), slice(512, NM)):
                            nc.tensor.matmul(ps_m[64:128, fs],
                                             atp[:, D * r:D * r + 64],
                                             ktb[:, NM * r + fs.start:
                                                  NM * r + fs.stop],
                                             start=(r == 0), stop=(r == 5),
                                             tile_position=(0, 64),
                                             skip_group_check=True)
                peb = sml.tile([128, NBLK], BF16, name="peb")
                nc.scalar.activation(peb[:], pdist[:], AF.Exp,
                                     bias=zero, scale=lbt)

                slot_tail(1, peb)

                # ---- partition-0 assembly ----
                def strided(row, col, *rest):
                    ap = frow[0:1, 128 * row + col:128 * row + col + 1]
                    return bass.AP(ap.tensor, ap.offset,
                                   [ap.ap[0], *rest])

                XXv = strided(1, PPC, [64, 2])
                XY0v = strided(2, PPC, [64, 2])
                YYv = strided(2, PPC + 1, [64, 2])
                sev = strided(3, PPC, [64, 2])
                # s0t = XX + YX + XY0 + YY in one grouped reduce
                quad = strided(1, PPC, [64, 2], [128, 2], [1, 2])
                s0t = sml.tile([1, 2], F32, name="s0t")
                nc.vector.tensor_reduce(s0t[:], quad,
                                        axis=mybir.AxisListType.XY,
                                        op=ALU.add)
                ck = sml.tile([1, 2], F32, name="ck")
                nc.vector.tensor_tensor(out=ck[:], in0=s0t[:], in1=sev,
                                        op=ALU.subtract)
                nc.vector.scalar_tensor_tensor(
                    out=ck[:], in0=ck[:], scalar=float(IC1), in1=aux4i,
                    op0=ALU.mult, op1=ALU.subtract)
                u1 = sml.tile([1, 2], F32, name="u1")
                nc.vector.tensor_tensor(out=u1[:], in0=XXv, in1=YYv,
                                        op=ALU.add)
                nc.vector.scalar_tensor_tensor(
                    out=u1[:], in0=u1[:], scalar=float(IC1), in1=aux4i,
                    op0=ALU.mult, op1=ALU.subtract)
                u2 = sml.tile([1, 2], F32, name="u2")
                nc.vector.tensor_tensor(out=u2[:], in0=XY0v, in1=sev,
                                        op=ALU.subtract)

                ubc = sml.tile([1, 2 * (PPC + 1)], F32, name="ubc")
                uf0 = ubc[0:1, 0:1]
                uF = bass.AP(uf0.tensor, uf0.offset,
                             [uf0.ap[0], [PPC + 1, 2]])
                nc.vector.scalar_tensor_tensor(
                    out=uF, in0=u2[:], scalar=float(-2.0 * IC2), in1=u1[:],
                    op0=ALU.mult, op1=ALU.add)
                ub0 = frow[0:1, 0:1]
                ub_src = bass.AP(ub0.tensor, ub0.offset,
                                 [ub0.ap[0], [64, 2], [1, PPC]])
                ubd0 = ubc[0:1, 1:2]
                ub_dst = bass.AP(ubd0.tensor, ubd0.offset,
                                 [ubd0.ap[0], [PPC + 1, 2], [1, PPC]])
                ckap = ck[0:1, 0:2]
                ck_b = bass.AP(ckap.tensor, ckap.offset,
                               [ckap.ap[0], [1, 2], [0, PPC]])
                nc.vector.tensor_tensor(out=ub_dst, in0=ub_src, in1=ck_b,
                                        op=ALU.add)
                nc.sync.dma_start(
                    out=out_d[:, :],
                    in_=ubc[0:1, :].rearrange("o (k p) -> o k p",
                                              p=PPC + 1))

    nc.compile()
    return nc


def _host_prep(X, Y, bandwidths, perms):
    X = np.ascontiguousarray(X, np.float32)
    Y = np.ascontiguousarray(Y, np.float32)
    perms = np.ascontiguousarray(perms, np.int32)
    Zf = np.concatenate([X, Y], 0)
    Zt = Zf.T.astype(np.float32)
    sq = (Zf.astype(np.float64) ** 2).sum(1).astype(np.float32)
    b = np.asarray(bandwidths, np.float64)

    zlr = np.zeros((D + 1, 2 * NM), np.float32)
    zlr[0:D, NM:] = Zt
    zlr[D, NM:] = 1.0
    zlr[0:D, 0:NM] = -2.0 * Zt
    zlr[D, 0:NM] = sq + BIAS

    idm = np.tile(np.eye(64, dtype=np.float32), (2, 1))

    maps = []
    for cid in range(NC):
        ka, kb = (0, 1) if cid < 4 else (2, 3)
        q = cid % 4
        pm = perms[q * PPC:(q + 1) * PPC]

        A = np.zeros((ROWS, NM), np.float32)
        A[np.arange(PPC)[:, None], pm[:, :N]] = 1
        A[PPC, :N] = 1
        A[PPC + 1, N:] = 1
        astk = np.zeros((128, NM), np.float32)
        astk[0:ROWS] = A
        astk[64:64 + ROWS] = A
        atp = np.zeros((128, 6 * D), np.float32)
        for c in range(6):
            atp[:, D * c:D * c + ROWS] = A[:, 128 * c:128 * (c + 1)].T
        A1 = A[:PPC, :N]
        A2 = A[:PPC, N:]
        Wc = (-KAP * (A1 * A2) + CB1 * A1 + CB2 * A2).astype(np.float32)
        wct = np.zeros((128, 3 * D), np.float32)
        for c in range(3):
            wct[:, D * c:D * c + PPC] = Wc[:, 128 * c:128 * (c + 1)].T
        bfp = np.zeros((128, NBLK * 4 + NM + 6 * D + 3 * D), np.float32)
        PB = NBLK * 4
        bfp[:, PB:PB + NM] = astk
        bfp[:, PB + NM:PB + NM + 6 * D] = atp
        bfp[:, PB + NM + 6 * D:PB + NM + 9 * D] = wct

        # pair partial squares: perm p pair j at lane (384p+j)%128, block
        # (384p+j)//128, 4 groups of 16 dims. Stripe pairs (j, 384+j) fill
        # blocks 3*PPC..3*PPC+2; stripe hits inside perm rows get a huge
        # sentinel so exp -> 0 (the zeroed K stripe).
        pX = pm[:, :N].astype(np.int64).ravel()
        pY = pm[:, N:].astype(np.int64).ravel()
        pdv = (Zf[pX] - Zf[pY]).astype(np.float32) ** 2
        psq = pdv.reshape(-1, 4, 16).sum(2) + np.float32(BIAS / 4)
        psq[pY == pX + N] = 1e6
        sdv = (Zf[:N] - Zf[N:]).astype(np.float32) ** 2
        psq = np.concatenate(
            [psq, sdv.reshape(-1, 4, 16).sum(2) + np.float32(BIAS / 4)], 0)
        psq = psq.reshape(NBLK, 128, 4).transpose(1, 0, 2).reshape(128, -1)
        bfp[:, 0:PB] = psq

        fsp = np.zeros((128, 32 + 64), np.float32)
        fsp[:, 32:96] = idm
        ga = np.float32(-1.0 / (b[ka] * b[ka]))
        lb = np.float32(-1.0 / b[kb])
        sqcols = sq.reshape(6, 128).T
        fsp[:, 0:6] = sqcols
        fsp[:, 6:12] = ga * sqcols
        fsp[:, 12] = ga
        fsp[:, 13] = lb
        fsp[:, 14] = 0.0
        d0a = np.exp(-BIAS / (b[ka] * b[ka]))
        d0b = np.exp(-np.sqrt(BIAS) / b[kb])
        fsp[0, 16] = np.float32(NM * d0a * IC1)
        fsp[0, 17] = np.float32(NM * d0b * IC1)

        maps.append(dict(zlr=zlr, bfp=bfp.astype(ml_dtypes.bfloat16),
                         fsp=fsp))
    return maps


_NC_CACHE = None


def _get_nc():
    global _NC_CACHE
    if _NC_CACHE is None:
        _NC_CACHE = _build()
    return _NC_CACHE


def _merge(results):
    full = np.zeros((4, 1 + NPER), np.float32)
    for cid in range(NC):
        ka, kb = (0, 1) if cid < 4 else (2, 3)
        q = cid % 4
        o = results[cid]["out"]
        full[ka, 1 + q * PPC:1 + (q + 1) * PPC] = o[0, 1:]
        full[kb, 1 + q * PPC:1 + (q + 1) * PPC] = o[1, 1:]
        if q == 0:
            full[ka, 0] = o[0, 0]
            full[kb, 0] = o[1, 0]
    return full


def kernel(X, Y, bandwidths, perms):
    nc = _get_nc()
    in_maps = _host_prep(X, Y, bandwidths, perms)
    res = bass_utils.run_bass_kernel_spmd(nc, in_maps, list(range(NC)))
    return _merge(res.results)


# revision 30
# speedup vs baseline: 1.1849x; 1.0245x over previous
"""Trainium2 Bass kernel for the 4-kernel MMD permutation test (nn_DUAL_78237124264373).

Sharding: 8 cores = 2 kernel-pairs x 4 permutation quarters. Core c<4 computes
kernels (0,1) [gaussian, laplacian] for perms [50*(c%4), 50*(c%4)+50); core
c>=4 the same for kernels (2,3). The host merges the [2, 1+50] per-core
outputs, so each core only ever evaluates TWO kernel matrices and the
activation-table sequence is exactly EXP (pre-warmed) -> SQRT -> EXP: the
swap points are pinned with zero-valued bias/scale tokens that data-depend
on the previous block's last op, so the Tile scheduler cannot interleave.

Per-core pipeline (slot a = gaussian, slot b = laplacian):
  d2 = L^T R on the PE in f32r (L = [Zt; 1], R = [-2 Zt; sq + B]), 12 PSUM
  pieces (6 row tiles x 512+256) in a 4-deep PSUM pool. Slot-a K = exp(ga*d2)
  straight out of PSUM with its M0 = A_aug K chunk matmul right behind, while
  the DVE lands clamped d2 in SBUF for the sqrt block. After the swap back to
  EXP, slot-b K = exp(lb*dist) runs chunk-by-chunk with M0 trailing. Each
  slot's row stats (aKa, aK1, colA), U_b vector and PE transpose into the
  partition-0 assembly row run as soon as that slot's M0 completes, so slot
  a's tail hides under the SQRT/slot-b window. U_b = KAP*(aKa - aK1) +
  W_corr @ e + (2/c2)*t + ck, with pair sums t reduced from host-gathered
  16-wide partial squares and e the K0[j, 384+j] stripe (3 extra pair
  blocks). The final scalar assembly (U, ck) happens on partition 0.
"""

import sys

import numpy as np

if "/opt/trn_rl_repo" not in sys.path:
    sys.path.insert(0, "/opt/trn_rl_repo")

import ml_dtypes

import concourse.bacc as bacc
import concourse.bass as bass
import concourse.mybir as mybir
import concourse.tile as tile
from concourse import bass_utils

N = 384
NM = 768
D = 64
NPER = 200
NC = 8
PPC = 50                      # perms per core
ROWS = PPC + 2                # + X-identity + Y-identity rows
NBLK = 3 * PPC + 3            # pair blocks of 128: 50 perms x 3 + stripe x 3
BIAS = 1e-3                   # keeps d2 > 0 under f32r rounding (see d0c)
C1 = float(N * (N - 1))
C2 = float(N * N)
KAP = np.float32(2.0 / C1 + 2.0 / C2)
CB1 = np.float32(1.0 / C1 + 2.0 / C2)
CB2 = np.float32(1.0 / C1)
TCO = np.float32(2.0 / C2)
IC1 = np.float32(1.0 / C1)
IC2 = np.float32(1.0 / C2)

F32 = mybir.dt.float32
F32R = mybir.dt.float32r
BF16 = mybir.dt.bfloat16
AF = mybir.ActivationFunctionType
ALU = mybir.AluOpType


def _build():
    nc = bacc.Bacc("TRN2", target_bir_lowering=False, debug=False)
    with tile.TileContext(nc) as tc:
        with tc.tile_pool(name="dram", bufs=1, space="DRAM") as dram, \
             tc.tile_pool(name="io", bufs=1) as io, \
             tc.tile_pool(name="big", bufs=1) as big, \
             tc.tile_pool(name="scr", bufs=1) as scr, \
             tc.tile_pool(name="sml", bufs=1) as sml:

            def din(name, shape, dt=F32):
                return dram.tile(shape, dt, kind="ExternalInput", name=name,
                                 uniquify=False)

            zlr_d = din("zlr", [D + 1, 2 * NM], F32R)
            bfp_d = din("bfp", [128, NBLK * 4 + NM + 6 * D + 3 * D], BF16)
            fsp_d = din("fsp", [128, 32 + 64], F32)
            out_d = dram.tile([2, 1 + PPC], F32, kind="ExternalOutput",
                              name="out", uniquify=False)

            # ---- input DMAs; zlr is [R | L] so the first piece lands first
            fsp = io.tile([128, 32 + 64], F32, name="fsp_sb")
            zlr = io.tile([D + 1, 2 * NM], F32R, name="zlr_sb")
            nc.sync.dma_start(out=zlr[:, 0:896], in_=zlr_d[:, 0:896])
            nc.scalar.dma_start(out=fsp[:], in_=fsp_d[:])
            bfp = io.tile([128, NBLK * 4 + NM + 6 * D + 3 * D], BF16,
                          name="bfp_sb")
            nc.gpsimd.dma_start(out=bfp[:], in_=bfp_d[:])
            nc.sync.dma_start(out=zlr[:, 896:], in_=zlr_d[:, 896:])

            zr = zlr[:, 0:NM]
            zl = zlr[:, NM:2 * NM]
            idm = fsp[:, 32:96]
            psq = bfp[:, 0:NBLK * 4]
            PB = NBLK * 4
            astk = bfp[:, PB:PB + NM]                # A rows at 0-51 / 64-115
            atp = bfp[:, PB + NM:PB + NM + 6 * D]    # A^T chunks, 64-padded
            wct = bfp[:, PB + NM + 6 * D:PB + NM + 9 * D]
            sqc = fsp[:, 0:6]                        # sq columns per row tile
            gbias = fsp[:, 6:12]                     # ga * sq per row tile
            ga = fsp[:, 12:13]
            lb = fsp[:, 13:14]
            zero = fsp[:, 14:15]
            aux4i = fsp[0:1, 16:18]                  # 768*d0c*IC1 per slot

            ones = io.tile([128, 1], F32, name="ones_sb")
            nc.vector.memset(ones[:], 1.0)
            onesb = io.tile([128, 1], BF16, name="onesb_sb")
            nc.vector.memset(onesb[:], 1.0)

            d2sb = big.tile([128, 6 * NM], F32, name="d2sb")
            dist = big.tile([128, 6 * NM], F32, name="dist_sb")
            kta = big.tile([128, 6 * NM], BF16, name="kta")
            ktb = big.tile([128, 6 * NM], BF16, name="ktb")
            M0sb = big.tile([128, NM], F32, name="M0sb")
            sA = scr.tile([128, N], F32, name="sA")
            sB = scr.tile([128, NM], F32, name="sB")
            pair2 = sml.tile([128, NBLK], F32, name="pair2")
            arow = sml.tile([128, 1], F32, name="arow")
            colA = sml.tile([128, 1], F32, name="colA")
            q0c = sml.tile([128, 1], F32, name="q0c")
            pack = sml.tile([128, 4], F32, name="pack")
            frow = sml.tile([1, 512], F32, name="frow")

            # warm the EXP activation table while DMAs are in flight
            warm = sml.tile([128, 1], F32, name="warm")
            nc.scalar.activation(warm[0:1, :], ones[0:1, :], AF.Exp,
                                 bias=0.0, scale=1.0)

            # ---- pair d2: reduce the host 16-wide partial squares ----
            psq3 = psq.rearrange("p (b d) -> p b d", d=4)
            nc.vector.tensor_reduce(pair2[:], psq3[:],
                                    axis=mybir.AxisListType.X, op=ALU.add)

            # ---- d2 phase: f32r matmuls, one [128,768] PSUM tile per row
            # tile, 3 deep; slot-a exp from PSUM; DVE lands d2 for sqrt ----
            with tc.tile_pool(name="psA", bufs=4, space="PSUM") as psA:
                for r in range(6):
                    lhs = zl[:, 128 * r:128 * (r + 1)]
                    ps_d2 = psA.tile([128, NM], F32, tag="d2",
                                     name=f"ps_d2_{r}")
                    nc.tensor.matmul(ps_d2[:, 0:512], lhs, zr[:, 0:512],
                                     start=True, stop=True)
                    nc.tensor.matmul(ps_d2[:, 512:NM], lhs, zr[:, 512:NM],
                                     start=True, stop=True)
                    sl = slice(NM * r, NM * (r + 1))
                    nc.scalar.activation(kta[:, sl], ps_d2[:], AF.Exp,
                                         scale=ga, bias=gbias[:, r:r + 1])
                    nc.vector.tensor_scalar(
                        out=d2sb[:, sl], in0=ps_d2[:],
                        scalar1=sqc[:, r:r + 1], scalar2=0.0,
                        op0=ALU.add, op1=ALU.max)


            with tc.tile_pool(name="psB", bufs=1, space="PSUM") as psB, \
                 tc.tile_pool(name="psC", bufs=1, space="PSUM") as psC:

                ps_m = psB.tile([128, NM], F32, name="ps_m")
                ps_tc = psC.tile([128, 2], F32, name="ps_tc")
                ps_t = ps_tc[:, 0:1]
                ps_corr = ps_tc[:, 1:2]
                ps_row = ps_m[0:1, 0:512]

                # slot-a pair exp rides the warm EXP table before the swap
                pea = sml.tile([128, NBLK], BF16, name="pea")
                nc.scalar.activation(pea[:], pair2[:], AF.Exp,
                                     bias=zero, scale=ga)
                # zb: zero bias that depends on the last EXP-block op, so
                # the scheduler cannot move the SQRT block earlier
                zb1 = sml.tile([128, 1], F32, name="zb1")
                nc.gpsimd.tensor_scalar(
                    out=zb1[:], in0=kta[:, 6 * NM - 1:6 * NM],
                    scalar1=0.0, scalar2=0.0, op0=ALU.mult, op1=ALU.add)
                zb = sml.tile([128, 1], F32, name="zb")
                nc.gpsimd.tensor_tensor(out=zb[:], in0=zb1[:],
                                        in1=pea[:, 0:1], op=ALU.mult)

                # slot-a M0 runs after the whole d2 phase (in the idle PE
                # sqrt window): interleaved, each d2 matmul would chain
                # behind the previous exp in the PE's in-order queue. The
                # zb-gated copy of atp pins the ordering.
                atp2 = scr.tile([128, 6 * D], BF16, name="atp2")
                nc.gpsimd.tensor_scalar(
                    out=atp2[:], in0=atp[:], scalar1=1.0, scalar2=zb[:],
                    op0=ALU.mult, op1=ALU.add)
                for r in range(6):
                    for fs in (slice(0, 512), slice(512, NM)):
                        nc.tensor.matmul(ps_m[0:64, fs],
                                         atp2[:, D * r:D * r + 64],
                                         kta[:, NM * r + fs.start:
                                              NM * r + fs.stop],
                                         start=(r == 0), stop=(r == 5),
                                         tile_position=(0, 0),
                                         skip_group_check=True)

                # ---- swap to SQRT: dist halves, then pair dist ----
                for h in range(2):
                    hs = slice(3 * NM * h, 3 * NM * (h + 1))
                    nc.scalar.activation(dist[:, hs], d2sb[:, hs], AF.Sqrt,
                                         bias=zb, scale=1.0)
                zb2 = sml.tile([128, 1], F32, name="zb2")
                nc.gpsimd.tensor_scalar(
                    out=zb2[:], in0=dist[:, 6 * NM - 1:6 * NM],
                    scalar1=0.0, scalar2=0.0, op0=ALU.mult, op1=ALU.add)
                pdist = sml.tile([128, NBLK], F32, name="pdist")
                nc.scalar.activation(pdist[:], pair2[:], AF.Sqrt,
                                     bias=zb2, scale=1.0)
                # lbt == lb, but depends on the last SQRT-block op
                lbt = sml.tile([128, 1], F32, name="lbt")
                nc.gpsimd.tensor_scalar(
                    out=lbt[:], in0=pdist[:, 0:1], scalar1=0.0,
                    scalar2=lb, op0=ALU.mult, op1=ALU.add)

                def slot_tail(i, pe):
                    """Pair sums, corrections, row stats, ubv and the PE
                    transpose for slot i; runs as soon as its M0 stops."""
                    pt = slice(64 * i, 64 * i + 64)
                    # t3: per-perm 3-block partial sums; group PPC holds the
                    # stripe so t[50] = sum(e) lands in ps_t for free
                    pe3 = pe.rearrange("p (g t) -> p g t", t=3)
                    t3 = sml.tile([128, 64], BF16, name=f"t3_{i}")
                    nc.vector.memset(t3[:, PPC + 1:64], 0.0)
                    with nc.allow_low_precision(reason="3-wide bf16 sum"):
                        nc.vector.tensor_reduce(t3[:, 0:PPC + 1], pe3[:],
                                                axis=mybir.AxisListType.X,
                                                op=ALU.add)
                    nc.tensor.matmul(ps_t[pt, :], t3[:], onesb[:],
                                     start=True, stop=True,
                                     tile_position=(0, 64 * i),
                                     skip_group_check=True)
                    for c in range(3):
                        nc.tensor.matmul(
                            ps_corr[pt, :], wct[:, D * c:D * c + 64],
                            pe[:, 3 * PPC + c:3 * PPC + c + 1],
                            start=(c == 0), stop=(c == 2),
                            tile_position=(0, 64 * i),
                            skip_group_check=True)
                    # row stats off this slot's half of ps_m; slot b uses
                    # the (idle) Scalar engine for the copy+rowsum so only
                    # sA rides the DVE tail; q0 reads PSUM directly
                    nc.vector.scalar_tensor_tensor(
                        out=sB[pt, :], in0=ps_m[pt, :], scalar=1.0,
                        in1=astk[pt, :], op0=ALU.mult, op1=ALU.mult,
                        accum_out=q0c[pt, :])
                    nc.vector.tensor_scalar(
                        out=sA[pt, :], in0=ps_m[pt, 0:N], scalar1=1.0,
                        scalar2=0.0, op0=ALU.mult, op1=ALU.add,
                        accum_out=pack[pt, 1:2])
                    if i == 0:
                        nc.vector.tensor_scalar(
                            out=M0sb[pt, :], in0=ps_m[pt, :], scalar1=1.0,
                            scalar2=0.0, op0=ALU.mult, op1=ALU.add,
                            accum_out=arow[pt, :])
                    else:
                        nc.scalar.activation(M0sb[pt, :], ps_m[pt, :],
                                             AF.Identity, bias=zero[pt],
                                             scale=1.0,
                                             accum_out=arow[pt, :])
                    # ubv = KAP*(q0 - arow) + corr + TCO*t into pack col 0;
                    # colA / colB = arow - colA / t into cols 1-3
                    nc.vector.tensor_tensor(out=pack[pt, 0:1], in0=q0c[pt, :],
                                            in1=arow[pt, :], op=ALU.subtract)
                    nc.vector.scalar_tensor_tensor(
                        out=pack[pt, 0:1], in0=pack[pt, 0:1],
                        scalar=float(KAP), in1=ps_corr[pt, :],
                        op0=ALU.mult, op1=ALU.add)
                    nc.vector.scalar_tensor_tensor(
                        out=pack[pt, 0:1], in0=ps_t[pt, :],
                        scalar=float(TCO), in1=pack[pt, 0:1],
                        op0=ALU.mult, op1=ALU.add)
                    nc.vector.tensor_copy(pack[pt, 3:4], ps_t[pt, :])
                    nc.vector.tensor_tensor(out=pack[pt, 2:3],
                                            in0=arow[pt, :],
                                            in1=pack[pt, 1:2],
                                            op=ALU.subtract)
                    # transpose the 4 pack columns into the partition-0 row
                    # (ps_m bank 0 is free again: stats above read it first)
                    for k in range(4):
                        nc.tensor.matmul(
                            ps_row[0:1,
                                   128 * k + 64 * i:128 * k + 64 * i + 64],
                            pack[pt, k:k + 1], idm[pt, :],
                            is_transpose=True, start=True, stop=True,
                            tile_position=(64 * i, 0),
                            skip_group_check=True)
                    s0 = ps_row[0:1, 64 * i:64 * i + 1]
                    f0 = frow[0:1, 64 * i:64 * i + 1]
                    nc.vector.tensor_copy(
                        bass.AP(f0.tensor, f0.offset,
                                [f0.ap[0], [128, 4], [1, 64]]),
                        bass.AP(s0.tensor, s0.offset,
                                [s0.ap[0], [128, 4], [1, 64]]))

                slot_tail(0, pea)

                # ---- swap back to EXP: slot-b K chunks + pair exp ----
                for c in range(3):
                    cs = slice(2 * NM * c, 2 * NM * (c + 1))
                    nc.scalar.activation(ktb[:, cs], dist[:, cs], AF.Exp,
                                         scale=lbt, bias=zero)
                    for r in (2 * c, 2 * c + 1):
                        for fs in (slice(0, 512), slice(512, NM)):
                            nc.tensor.matmul(ps_m[64:128, fs],
                                             atp[:, D * r:D * r + 64],
                                             ktb[:, NM * r + fs.start:
                                                  NM * r + fs.stop],
                                             start=(r == 0), stop=(r == 5),
                                             tile_position=(0, 64),
                                             skip_group_check=True)
                peb = sml.tile([128, NBLK], BF16, name="peb")
                nc.scalar.activation(peb[:], pdist[:], AF.Exp,
                                     bias=zero, scale=lbt)

                slot_tail(1, peb)

                # ---- partition-0 assembly ----
                def strided(row, col, *rest):
                    ap = frow[0:1, 128 * row + col:128 * row + col + 1]
                    return bass.AP(ap.tensor, ap.offset,
                                   [ap.ap[0], *rest])

                XXv = strided(1, PPC, [64, 2])
                XY0v = strided(2, PPC, [64, 2])
                YYv = strided(2, PPC + 1, [64, 2])
                sev = strided(3, PPC, [64, 2])
                # s0t = XX + YX + XY0 + YY in one grouped reduce
                quad = strided(1, PPC, [64, 2], [128, 2], [1, 2])
                s0t = sml.tile([1, 2], F32, name="s0t")
                nc.vector.tensor_reduce(s0t[:], quad,
                                        axis=mybir.AxisListType.XY,
                                        op=ALU.add)
                ck = sml.tile([1, 2], F32, name="ck")
                nc.vector.tensor_tensor(out=ck[:], in0=s0t[:], in1=sev,
                                        op=ALU.subtract)
                nc.vector.scalar_tensor_tensor(
                    out=ck[:], in0=ck[:], scalar=float(IC1), in1=aux4i,
                    op0=ALU.mult, op1=ALU.subtract)
                u1 = sml.tile([1, 2], F32, name="u1")
                nc.vector.tensor_tensor(out=u1[:], in0=XXv, in1=YYv,
                                        op=ALU.add)
                nc.vector.scalar_tensor_tensor(
                    out=u1[:], in0=u1[:], scalar=float(IC1), in1=aux4i,
                    op0=ALU.mult, op1=ALU.subtract)
                u2 = sml.tile([1, 2], F32, name="u2")
                nc.vector.tensor_tensor(out=u2[:], in0=XY0v, in1=sev,
                                        op=ALU.subtract)

                ubc = sml.tile([1, 2 * (PPC + 1)], F32, name="ubc")
                uf0 = ubc[0:1, 0:1]
                uF = bass.AP(uf0.tensor, uf0.offset,
                             [uf0.ap[0], [PPC + 1, 2]])
                nc.vector.scalar_tensor_tensor(
                    out=uF, in0=u2[:], scalar=float(-2.0 * IC2), in1=u1[:],
                    op0=ALU.mult, op1=ALU.add)
                ub0 = frow[0:1, 0:1]
                ub_src = bass.AP(ub0.tensor, ub0.offset,
                                 [ub0.ap[0], [64, 2], [1, PPC]])
                ubd0 = ubc[0:1, 1:2]
                ub_dst = bass.AP(ubd0.tensor, ubd0.offset,
                                 [ubd0.ap[0], [PPC + 1, 2], [1, PPC]])
                ckap = ck[0:1, 0:2]
                ck_b = bass.AP(ckap.tensor, ckap.offset,
                               [ckap.ap[0], [1, 2], [0, PPC]])
                nc.vector.tensor_tensor(out=ub_dst, in0=ub_src, in1=ck_b,
                                        op=ALU.add)
                nc.sync.dma_start(
                    out=out_d[:, :],
                    in_=ubc[0:1, :].rearrange("o (k p) -> o k p",
                                              p=PPC + 1))

    nc.compile()
    return nc


def _host_prep(X, Y, bandwidths, perms):
    X = np.ascontiguousarray(X, np.float32)
    Y = np.ascontiguousarray(Y, np.float32)
    perms = np.ascontiguousarray(perms, np.int32)
    Zf = np.concatenate([X, Y], 0)
    Zt = Zf.T.astype(np.float32)
    sq = (Zf.astype(np.float64) ** 2).sum(1).astype(np.float32)
    b = np.asarray(bandwidths, np.float64)

    zlr = np.zeros((D + 1, 2 * NM), np.float32)
    zlr[0:D, NM:] = Zt
    zlr[D, NM:] = 1.0
    zlr[0:D, 0:NM] = -2.0 * Zt
    zlr[D, 0:NM] = sq + BIAS

    idm = np.tile(np.eye(64, dtype=np.float32), (2, 1))

    maps = []
    for cid in range(NC):
        ka, kb = (0, 1) if cid < 4 else (2, 3)
        q = cid % 4
        pm = perms[q * PPC:(q + 1) * PPC]

        A = np.zeros((ROWS, NM), np.float32)
        A[np.arange(PPC)[:, None], pm[:, :N]] = 1
        A[PPC, :N] = 1
        A[PPC + 1, N:] = 1
        astk = np.zeros((128, NM), np.float32)
        astk[0:ROWS] = A
        astk[64:64 + ROWS] = A
        atp = np.zeros((128, 6 * D), np.float32)
        for c in range(6):
            atp[:, D * c:D * c + ROWS] = A[:, 128 * c:128 * (c + 1)].T
        A1 = A[:PPC, :N]
        A2 = A[:PPC, N:]
        Wc = (-KAP * (A1 * A2) + CB1 * A1 + CB2 * A2).astype(np.float32)
        wct = np.zeros((128, 3 * D), np.float32)
        for c in range(3):
            wct[:, D * c:D * c + PPC] = Wc[:, 128 * c:128 * (c + 1)].T
        bfp = np.zeros((128, NBLK * 4 + NM + 6 * D + 3 * D), np.float32)
        PB = NBLK * 4
        bfp[:, PB:PB + NM] = astk
        bfp[:, PB + NM:PB + NM + 6 * D] = atp
        bfp[:, PB + NM + 6 * D:PB + NM + 9 * D] = wct

        # pair partial squares: perm p pair j at lane (384p+j)%128, block
        # (384p+j)//128, 4 groups of 16 dims. Stripe pairs (j, 384+j) fill
        # blocks 3*PPC..3*PPC+2; stripe hits inside perm rows get a huge
        # sentinel so exp -> 0 (the zeroed K stripe).
        pX = pm[:, :N].astype(np.int64).ravel()
        pY = pm[:, N:].astype(np.int64).ravel()
        pdv = (Zf[pX] - Zf[pY]).astype(np.float32) ** 2
        psq = pdv.reshape(-1, 4, 16).sum(2) + np.float32(BIAS / 4)
        psq[pY == pX + N] = 1e6
        sdv = (Zf[:N] - Zf[N:]).astype(np.float32) ** 2
        psq = np.concatenate(
            [psq, sdv.reshape(-1, 4, 16).sum(2) + np.float32(BIAS / 4)], 0)
        psq = psq.reshape(NBLK, 128, 4).transpose(1, 0, 2).reshape(128, -1)
        bfp[:, 0:PB] = psq

        fsp = np.zeros((128, 32 + 64), np.float32)
        fsp[:, 32:96] = idm
        ga = np.float32(-1.0 / (b[ka] * b[ka]))
        lb = np.float32(-1.0 / b[kb])
        sqcols = sq.reshape(6, 128).T
        fsp[:, 0:6] = sqcols
        fsp[:, 6:12] = ga * sqcols
        fsp[:, 12] = ga
        fsp[:, 13] = lb
        fsp[:, 14] = 0.0
        d0a = np.exp(-BIAS / (b[ka] * b[ka]))
        d0b = np.exp(-np.sqrt(BIAS) / b[kb])
        fsp[0, 16] = np.float32(NM * d0a * IC1)
        fsp[0, 17] = np.float32(NM * d0b * IC1)

        maps.append(dict(zlr=zlr, bfp=bfp.astype(ml_dtypes.bfloat16),
                         fsp=fsp))
    return maps


_NC_CACHE = None


def _get_nc():
    global _NC_CACHE
    if _NC_CACHE is None:
        _NC_CACHE = _build()
    return _NC_CACHE


def _merge(results):
    full = np.zeros((4, 1 + NPER), np.float32)
    for cid in range(NC):
        ka, kb = (0, 1) if cid < 4 else (2, 3)
        q = cid % 4
        o = results[cid]["out"]
        full[ka, 1 + q * PPC:1 + (q + 1) * PPC] = o[0, 1:]
        full[kb, 1 + q * PPC:1 + (q + 1) * PPC] = o[1, 1:]
        if q == 0:
            full[ka, 0] = o[0, 0]
            full[kb, 0] = o[1, 0]
    return full


def kernel(X, Y, bandwidths, perms):
    nc = _get_nc()
    in_maps = _host_prep(X, Y, bandwidths, perms)
    res = bass_utils.run_bass_kernel_spmd(nc, in_maps, list(range(NC)))
    return _merge(res.results)


# revision 33
# speedup vs baseline: 1.1907x; 1.0048x over previous
"""Trainium2 Bass kernel for the 4-kernel MMD permutation test (nn_DUAL_78237124264373).

Sharding: 8 cores = 2 kernel-pairs x 4 permutation quarters. Core c<4 computes
kernels (0,1) [gaussian, laplacian] for perms [50*(c%4), 50*(c%4)+50); core
c>=4 the same for kernels (2,3). The host merges the [2, 1+50] per-core
outputs, so each core only ever evaluates TWO kernel matrices and the
activation-table sequence is exactly EXP (pre-warmed) -> SQRT -> EXP: the
swap points are pinned with zero-valued bias/scale tokens that data-depend
on the previous block's last op, so the Tile scheduler cannot interleave.

Per-core pipeline (slot a = gaussian, slot b = laplacian):
  d2 = L^T R on the PE in f32r (L = [Zt; 1], R = [-2 Zt; sq + B]), 12 PSUM
  pieces (6 row tiles x 512+256) in a 4-deep PSUM pool. Slot-a K = exp(ga*d2)
  straight out of PSUM with its M0 = A_aug K chunk matmul right behind, while
  the DVE lands clamped d2 in SBUF for the sqrt block. After the swap back to
  EXP, slot-b K = exp(lb*dist) runs chunk-by-chunk with M0 trailing. Each
  slot's row stats (aKa, aK1, colA), U_b vector and PE transpose into the
  partition-0 assembly row run as soon as that slot's M0 completes, so slot
  a's tail hides under the SQRT/slot-b window. U_b = KAP*(aKa - aK1) +
  W_corr @ e + (2/c2)*t + ck, with pair sums t reduced from host-gathered
  16-wide partial squares and e the K0[j, 384+j] stripe (3 extra pair
  blocks). The final scalar assembly (U, ck) happens on partition 0.
"""

import sys

import numpy as np

if "/opt/trn_rl_repo" not in sys.path:
    sys.path.insert(0, "/opt/trn_rl_repo")

import ml_dtypes

import concourse.bacc as bacc
import concourse.bass as bass
import concourse.mybir as mybir
import concourse.tile as tile
from concourse import bass_utils

N = 384
NM = 768
D = 64
NPER = 200
NC = 8
PPC = 50                      # perms per core
ROWS = PPC + 2                # + X-identity + Y-identity rows
NBLK = 3 * PPC + 3            # pair blocks of 128: 50 perms x 3 + stripe x 3
BIAS = 1e-3                   # keeps d2 > 0 under f32r rounding (see d0c)
C1 = float(N * (N - 1))
C2 = float(N * N)
KAP = np.float32(2.0 / C1 + 2.0 / C2)
CB1 = np.float32(1.0 / C1 + 2.0 / C2)
CB2 = np.float32(1.0 / C1)
TCO = np.float32(2.0 / C2)
IC1 = np.float32(1.0 / C1)
IC2 = np.float32(1.0 / C2)

F32 = mybir.dt.float32
F32R = mybir.dt.float32r
BF16 = mybir.dt.bfloat16
AF = mybir.ActivationFunctionType
ALU = mybir.AluOpType


def _build():
    nc = bacc.Bacc("TRN2", target_bir_lowering=False, debug=False)
    with tile.TileContext(nc) as tc:
        with tc.tile_pool(name="dram", bufs=1, space="DRAM") as dram, \
             tc.tile_pool(name="io", bufs=1) as io, \
             tc.tile_pool(name="big", bufs=1) as big, \
             tc.tile_pool(name="scr", bufs=1) as scr, \
             tc.tile_pool(name="sml", bufs=1) as sml:

            def din(name, shape, dt=F32):
                return dram.tile(shape, dt, kind="ExternalInput", name=name,
                                 uniquify=False)

            zlr_d = din("zlr", [D + 1, 2 * NM], F32R)
            bfp_d = din("bfp", [128, NBLK * 4 + NM + 6 * D + 3 * D], BF16)
            fsp_d = din("fsp", [128, 32 + 64], F32)
            out_d = dram.tile([2, 1 + PPC], F32, kind="ExternalOutput",
                              name="out", uniquify=False)

            # ---- input DMAs; zlr is [R | L] so the first piece lands first
            fsp = io.tile([128, 32 + 64], F32, name="fsp_sb")
            zlr = io.tile([D + 1, 2 * NM], F32R, name="zlr_sb")
            nc.sync.dma_start(out=zlr[:, 0:896], in_=zlr_d[:, 0:896])
            nc.scalar.dma_start(out=fsp[:], in_=fsp_d[:])
            nc.gpsimd.dma_start(out=zlr[:, 896:], in_=zlr_d[:, 896:])
            bfp = io.tile([128, NBLK * 4 + NM + 6 * D + 3 * D], BF16,
                          name="bfp_sb")
            nc.sync.dma_start(out=bfp[:], in_=bfp_d[:])

            zr = zlr[:, 0:NM]
            zl = zlr[:, NM:2 * NM]
            idm = fsp[:, 32:96]
            psq = bfp[:, 0:NBLK * 4]
            PB = NBLK * 4
            astk = bfp[:, PB:PB + NM]                # A rows at 0-51 / 64-115
            atp = bfp[:, PB + NM:PB + NM + 6 * D]    # A^T chunks, 64-padded
            wct = bfp[:, PB + NM + 6 * D:PB + NM + 9 * D]
            sqc = fsp[:, 0:6]                        # sq columns per row tile
            gbias = fsp[:, 6:12]                     # ga * sq per row tile
            ga = fsp[:, 12:13]
            lb = fsp[:, 13:14]
            zero = fsp[:, 14:15]
            aux4i = fsp[0:1, 16:18]                  # 768*d0c*IC1 per slot

            ones = io.tile([128, 1], F32, name="ones_sb")
            nc.vector.memset(ones[:], 1.0)
            onesb = io.tile([128, 1], BF16, name="onesb_sb")
            nc.vector.memset(onesb[:], 1.0)

            d2sb = big.tile([128, 6 * NM], F32, name="d2sb")
            dist = big.tile([128, 6 * NM], F32, name="dist_sb")
            kta = big.tile([128, 6 * NM], BF16, name="kta")
            ktb = big.tile([128, 6 * NM], BF16, name="ktb")
            M0sb = big.tile([128, NM], F32, name="M0sb")
            sA = scr.tile([128, N], F32, name="sA")
            sB = scr.tile([128, NM], F32, name="sB")
            pair2 = sml.tile([128, NBLK], F32, name="pair2")
            arow = sml.tile([128, 1], F32, name="arow")
            colA = sml.tile([128, 1], F32, name="colA")
            q0c = sml.tile([128, 1], F32, name="q0c")
            pack = sml.tile([128, 4], F32, name="pack")
            frow = sml.tile([1, 512], F32, name="frow")
            ubc = sml.tile([1, 2 * (PPC + 1)], F32, name="ubc")

            # warm the EXP activation table while DMAs are in flight
            warm = sml.tile([128, 1], F32, name="warm")
            nc.scalar.activation(warm[0:1, :], ones[0:1, :], AF.Exp,
                                 bias=0.0, scale=1.0)

            # ---- pair d2: reduce the host 16-wide partial squares ----
            psq3 = psq.rearrange("p (b d) -> p b d", d=4)
            nc.vector.tensor_reduce(pair2[:], psq3[:],
                                    axis=mybir.AxisListType.X, op=ALU.add)

            # ---- d2 phase: f32r matmuls, one [128,768] PSUM tile per row
            # tile, 3 deep; slot-a exp from PSUM; DVE lands d2 for sqrt ----
            with tc.tile_pool(name="psA", bufs=4, space="PSUM") as psA:
                for r in range(6):
                    lhs = zl[:, 128 * r:128 * (r + 1)]
                    ps_d2 = psA.tile([128, NM], F32, tag="d2",
                                     name=f"ps_d2_{r}")
                    nc.tensor.matmul(ps_d2[:, 0:512], lhs, zr[:, 0:512],
                                     start=True, stop=True)
                    nc.tensor.matmul(ps_d2[:, 512:NM], lhs, zr[:, 512:NM],
                                     start=True, stop=True)
                    sl = slice(NM * r, NM * (r + 1))
                    nc.scalar.activation(kta[:, sl], ps_d2[:], AF.Exp,
                                         scale=ga, bias=gbias[:, r:r + 1])
                    nc.vector.tensor_scalar(
                        out=d2sb[:, sl], in0=ps_d2[:],
                        scalar1=sqc[:, r:r + 1], scalar2=0.0,
                        op0=ALU.add, op1=ALU.max)


            with tc.tile_pool(name="psB", bufs=1, space="PSUM") as psB, \
                 tc.tile_pool(name="psC", bufs=1, space="PSUM") as psC:

                ps_m = psB.tile([128, NM], F32, name="ps_m")
                ps_tc = psC.tile([128, 2], F32, name="ps_tc")
                ps_t = ps_tc[:, 0:1]
                ps_corr = ps_tc[:, 1:2]
                ps_row = ps_m[0:1, 0:512]

                # slot-a pair exp rides the warm EXP table before the swap
                pea = sml.tile([128, NBLK], BF16, name="pea")
                nc.scalar.activation(pea[:], pair2[:], AF.Exp,
                                     bias=zero, scale=ga)
                # zb: zero bias that depends on the last EXP-block op, so
                # the scheduler cannot move the SQRT block earlier
                zb1 = sml.tile([128, 1], F32, name="zb1")
                nc.gpsimd.tensor_scalar(
                    out=zb1[:], in0=kta[:, 6 * NM - 1:6 * NM],
                    scalar1=0.0, scalar2=0.0, op0=ALU.mult, op1=ALU.add)
                zb = sml.tile([128, 1], F32, name="zb")
                nc.gpsimd.tensor_tensor(out=zb[:], in0=zb1[:],
                                        in1=pea[:, 0:1], op=ALU.mult)

                # slot-a M0 runs after the whole d2 phase (in the idle PE
                # sqrt window): interleaved, each d2 matmul would chain
                # behind the previous exp in the PE's in-order queue. The
                # zb-gated copy of atp pins the ordering.
                atp2 = scr.tile([128, 6 * D], BF16, name="atp2")
                nc.gpsimd.tensor_scalar(
                    out=atp2[:], in0=atp[:], scalar1=1.0, scalar2=zb[:],
                    op0=ALU.mult, op1=ALU.add)
                for r in range(6):
                    for fs in (slice(0, 512), slice(512, NM)):
                        nc.tensor.matmul(ps_m[0:64, fs],
                                         atp2[:, D * r:D * r + 64],
                                         kta[:, NM * r + fs.start:
                                              NM * r + fs.stop],
                                         start=(r == 0), stop=(r == 5),
                                         tile_position=(0, 0),
                                         skip_group_check=True)

                # ---- swap to SQRT: dist halves, then pair dist ----
                for h in range(2):
                    hs = slice(3 * NM * h, 3 * NM * (h + 1))
                    nc.scalar.activation(dist[:, hs], d2sb[:, hs], AF.Sqrt,
                                         bias=zb, scale=1.0)
                zb2 = sml.tile([128, 1], F32, name="zb2")
                nc.gpsimd.tensor_scalar(
                    out=zb2[:], in0=dist[:, 3 * NM - 1:3 * NM],
                    scalar1=0.0, scalar2=0.0, op0=ALU.mult, op1=ALU.add)
                pdist = sml.tile([128, NBLK], F32, name="pdist")
                nc.scalar.activation(pdist[:], pair2[:], AF.Sqrt,
                                     bias=zb2, scale=1.0)
                # lbt == lb, but depends on the last SQRT-block op
                lbt = sml.tile([128, 1], F32, name="lbt")
                nc.gpsimd.tensor_scalar(
                    out=lbt[:], in0=pdist[:, 0:1], scalar1=0.0,
                    scalar2=lb, op0=ALU.mult, op1=ALU.add)

                def slot_tail(i, pe):
                    """Pair sums, corrections, row stats, ubv and the PE
                    transpose for slot i; runs as soon as its M0 stops."""
                    pt = slice(64 * i, 64 * i + 64)
                    # t3: per-perm 3-block partial sums; group PPC holds the
                    # stripe so t[50] = sum(e) lands in ps_t for free
                    pe3 = pe.rearrange("p (g t) -> p g t", t=3)
                    t3 = sml.tile([128, 64], BF16, name=f"t3_{i}")
                    nc.vector.memset(t3[:, PPC + 1:64], 0.0)
                    with nc.allow_low_precision(reason="3-wide bf16 sum"):
                        nc.vector.tensor_reduce(t3[:, 0:PPC + 1], pe3[:],
                                                axis=mybir.AxisListType.X,
                                                op=ALU.add)
                    nc.tensor.matmul(ps_t[pt, :], t3[:], onesb[:],
                                     start=True, stop=True,
                                     tile_position=(0, 64 * i),
                                     skip_group_check=True)
                    for c in range(3):
                        nc.tensor.matmul(
                            ps_corr[pt, :], wct[:, D * c:D * c + 64],
                            pe[:, 3 * PPC + c:3 * PPC + c + 1],
                            start=(c == 0), stop=(c == 2),
                            tile_position=(0, 64 * i),
                            skip_group_check=True)
                    # row stats off this slot's half of ps_m; slot b uses
                    # the (idle) Scalar engine for the copy+rowsum so only
                    # sA rides the DVE tail; q0 reads PSUM directly
                    nc.vector.scalar_tensor_tensor(
                        out=sB[pt, :], in0=ps_m[pt, :], scalar=1.0,
                        in1=astk[pt, :], op0=ALU.mult, op1=ALU.mult,
                        accum_out=q0c[pt, :])
                    nc.vector.tensor_scalar(
                        out=sA[pt, :], in0=ps_m[pt, 0:N], scalar1=1.0,
                        scalar2=0.0, op0=ALU.mult, op1=ALU.add,
                        accum_out=pack[pt, 1:2])
                    if i == 0:
                        nc.vector.tensor_scalar(
                            out=M0sb[pt, :], in0=ps_m[pt, :], scalar1=1.0,
                            scalar2=0.0, op0=ALU.mult, op1=ALU.add,
                            accum_out=arow[pt, :])
                    else:
                        nc.scalar.activation(M0sb[pt, :], ps_m[pt, :],
                                             AF.Identity, bias=zero[pt],
                                             scale=1.0,
                                             accum_out=arow[pt, :])
                    # ubv = KAP*(q0 - arow) + corr + TCO*t into pack col 0;
                    # colA / colB = arow - colA / t into cols 1-3
                    nc.vector.tensor_tensor(out=pack[pt, 0:1], in0=q0c[pt, :],
                                            in1=arow[pt, :], op=ALU.subtract)
                    nc.vector.scalar_tensor_tensor(
                        out=pack[pt, 0:1], in0=pack[pt, 0:1],
                        scalar=float(KAP), in1=ps_corr[pt, :],
                        op0=ALU.mult, op1=ALU.add)
                    nc.vector.scalar_tensor_tensor(
                        out=pack[pt, 0:1], in0=ps_t[pt, :],
                        scalar=float(TCO), in1=pack[pt, 0:1],
                        op0=ALU.mult, op1=ALU.add)
                    nc.vector.tensor_copy(pack[pt, 3:4], ps_t[pt, :])
                    nc.vector.tensor_tensor(out=pack[pt, 2:3],
                                            in0=arow[pt, :],
                                            in1=pack[pt, 1:2],
                                            op=ALU.subtract)
                    # transpose the 4 pack columns into the partition-0 row
                    # (ps_m bank 0 is free again: stats above read it first)
                    for k in range(4):
                        nc.tensor.matmul(
                            ps_row[0:1,
                                   128 * k + 64 * i:128 * k + 64 * i + 64],
                            pack[pt, k:k + 1], idm[pt, :],
                            is_transpose=True, start=True, stop=True,
                            tile_position=(64 * i, 0),
                            skip_group_check=True)
                    s0 = ps_row[0:1, 64 * i:64 * i + 1]
                    f0 = frow[0:1, 64 * i:64 * i + 1]
                    nc.vector.tensor_copy(
                        bass.AP(f0.tensor, f0.offset,
                                [f0.ap[0], [128, 4], [1, 64]]),
                        bass.AP(s0.tensor, s0.offset,
                                [s0.ap[0], [128, 4], [1, 64]]))

                    # per-slot partition-0 assembly: U, ck, output row
                    def sv(row, col, *rest):
                        ap = frow[0:1, 128 * row + 64 * i + col:
                                  128 * row + 64 * i + col + 1]
                        return bass.AP(ap.tensor, ap.offset,
                                       [ap.ap[0], *(rest or ([1, 1],))])

                    XXv = sv(1, PPC)
                    XY0v = sv(2, PPC)
                    YYv = sv(2, PPC + 1)
                    sev = sv(3, PPC)
                    quad = sv(1, PPC, [128, 2], [1, 2])
                    s0t = sml.tile([1, 2], F32, name=f"s0t{i}")
                    nc.vector.tensor_reduce(s0t[0:1, 0:1], quad,
                                            axis=mybir.AxisListType.XY,
                                            op=ALU.add)
                    ck = sml.tile([1, 2], F32, name=f"ck{i}")
                    nc.vector.tensor_tensor(out=ck[0:1, 0:1],
                                            in0=s0t[0:1, 0:1], in1=sev,
                                            op=ALU.subtract)
                    nc.vector.scalar_tensor_tensor(
                        out=ck[0:1, 0:1], in0=ck[0:1, 0:1],
                        scalar=float(IC1), in1=aux4i[0:1, i:i + 1],
                        op0=ALU.mult, op1=ALU.subtract)
                    u1 = sml.tile([1, 2], F32, name=f"u1{i}")
                    nc.vector.tensor_tensor(out=u1[0:1, 0:1], in0=XXv,
                                            in1=YYv, op=ALU.add)
                    nc.vector.scalar_tensor_tensor(
                        out=u1[0:1, 0:1], in0=u1[0:1, 0:1],
                        scalar=float(IC1), in1=aux4i[0:1, i:i + 1],
                        op0=ALU.mult, op1=ALU.subtract)
                    u2 = sml.tile([1, 2], F32, name=f"u2{i}")
                    nc.vector.tensor_tensor(out=u2[0:1, 0:1], in0=XY0v,
                                            in1=sev, op=ALU.subtract)
                    nc.vector.scalar_tensor_tensor(
                        out=ubc[0:1, (PPC + 1) * i:(PPC + 1) * i + 1],
                        in0=u2[0:1, 0:1], scalar=float(-2.0 * IC2),
                        in1=u1[0:1, 0:1], op0=ALU.mult, op1=ALU.add)
                    ub0 = frow[0:1, 64 * i:64 * i + 1]
                    ub_src = bass.AP(ub0.tensor, ub0.offset,
                                     [ub0.ap[0], [1, PPC]])
                    ckap = ck[0:1, 0:1]
                    ck_b = bass.AP(ckap.tensor, ckap.offset,
                                   [ckap.ap[0], [0, PPC]])
                    nc.vector.tensor_tensor(
                        out=ubc[0:1, (PPC + 1) * i + 1:(PPC + 1) * (i + 1)],
                        in0=ub_src, in1=ck_b, op=ALU.add)

                slot_tail(0, pea)

                # ---- swap back to EXP: slot-b K chunks + pair exp ----
                for c in range(3):
                    cs = slice(2 * NM * c, 2 * NM * (c + 1))
                    nc.scalar.activation(ktb[:, cs], dist[:, cs], AF.Exp,
                                         scale=lbt, bias=zero)
                    for r in (2 * c, 2 * c + 1):
                        for fs in (slice(0, 512), slice(512, NM)):
                            nc.tensor.matmul(ps_m[64:128, fs],
                                             atp[:, D * r:D * r + 64],
                                             ktb[:, NM * r + fs.start:
                                                  NM * r + fs.stop],
                                             start=(r == 0), stop=(r == 5),
                                             tile_position=(0, 64),
                                             skip_group_check=True)
                peb = sml.tile([128, NBLK], BF16, name="peb")
                nc.scalar.activation(peb[:], pdist[:], AF.Exp,
                                     bias=zero, scale=lbt)

                slot_tail(1, peb)

                # output row was assembled per slot in slot_tail
                nc.sync.dma_start(
                    out=out_d[:, :],
                    in_=ubc[0:1, :].rearrange("o (k p) -> o k p",
                                              p=PPC + 1))

    nc.compile()
    return nc


def _host_prep(X, Y, bandwidths, perms):
    X = np.ascontiguousarray(X, np.float32)
    Y = np.ascontiguousarray(Y, np.float32)
    perms = np.ascontiguousarray(perms, np.int32)
    Zf = np.concatenate([X, Y], 0)
    Zt = Zf.T.astype(np.float32)
    sq = (Zf.astype(np.float64) ** 2).sum(1).astype(np.float32)
    b = np.asarray(bandwidths, np.float64)

    zlr = np.zeros((D + 1, 2 * NM), np.float32)
    zlr[0:D, NM:] = Zt
    zlr[D, NM:] = 1.0
    zlr[0:D, 0:NM] = -2.0 * Zt
    zlr[D, 0:NM] = sq + BIAS

    idm = np.tile(np.eye(64, dtype=np.float32), (2, 1))

    maps = []
    for cid in range(NC):
        ka, kb = (0, 1) if cid < 4 else (2, 3)
        q = cid % 4
        pm = perms[q * PPC:(q + 1) * PPC]

        A = np.zeros((ROWS, NM), np.float32)
        A[np.arange(PPC)[:, None], pm[:, :N]] = 1
        A[PPC, :N] = 1
        A[PPC + 1, N:] = 1
        astk = np.zeros((128, NM), np.float32)
        astk[0:ROWS] = A
        astk[64:64 + ROWS] = A
        atp = np.zeros((128, 6 * D), np.float32)
        for c in range(6):
            atp[:, D * c:D * c + ROWS] = A[:, 128 * c:128 * (c + 1)].T
        A1 = A[:PPC, :N]
        A2 = A[:PPC, N:]
        Wc = (-KAP * (A1 * A2) + CB1 * A1 + CB2 * A2).astype(np.float32)
        wct = np.zeros((128, 3 * D), np.float32)
        for c in range(3):
            wct[:, D * c:D * c + PPC] = Wc[:, 128 * c:128 * (c + 1)].T
        bfp = np.zeros((128, NBLK * 4 + NM + 6 * D + 3 * D), np.float32)
        PB = NBLK * 4
        bfp[:, PB:PB + NM] = astk
        bfp[:, PB + NM:PB + NM + 6 * D] = atp
        bfp[:, PB + NM + 6 * D:PB + NM + 9 * D] = wct

        # pair partial squares: perm p pair j at lane (384p+j)%128, block
        # (384p+j)//128, 4 groups of 16 dims. Stripe pairs (j, 384+j) fill
        # blocks 3*PPC..3*PPC+2; stripe hits inside perm rows get a huge
        # sentinel so exp -> 0 (the zeroed K stripe).
        pX = pm[:, :N].astype(np.int64).ravel()
        pY = pm[:, N:].astype(np.int64).ravel()
        pdv = (Zf[pX] - Zf[pY]).astype(np.float32) ** 2
        psq = pdv.reshape(-1, 4, 16).sum(2) + np.float32(BIAS / 4)
        psq[pY == pX + N] = 1e6
        sdv = (Zf[:N] - Zf[N:]).astype(np.float32) ** 2
        psq = np.concatenate(
            [psq, sdv.reshape(-1, 4, 16).sum(2) + np.float32(BIAS / 4)], 0)
        psq = psq.reshape(NBLK, 128, 4).transpose(1, 0, 2).reshape(128, -1)
        bfp[:, 0:PB] = psq

        fsp = np.zeros((128, 32 + 64), np.float32)
        fsp[:, 32:96] = idm
        ga = np.float32(-1.0 / (b[ka] * b[ka]))
        lb = np.float32(-1.0 / b[kb])
        sqcols = sq.reshape(6, 128).T
        fsp[:, 0:6] = sqcols
        fsp[:, 6:12] = ga * sqcols
        fsp[:, 12] = ga
        fsp[:, 13] = lb
        fsp[:, 14] = 0.0
        d0a = np.exp(-BIAS / (b[ka] * b[ka]))
        d0b = np.exp(-np.sqrt(BIAS) / b[kb])
        fsp[0, 16] = np.float32(NM * d0a * IC1)
        fsp[0, 17] = np.float32(NM * d0b * IC1)

        maps.append(dict(zlr=zlr, bfp=bfp.astype(ml_dtypes.bfloat16),
                         fsp=fsp))
    return maps


_NC_CACHE = None


def _get_nc():
    global _NC_CACHE
    if _NC_CACHE is None:
        _NC_CACHE = _build()
    return _NC_CACHE


def _merge(results):
    full = np.zeros((4, 1 + NPER), np.float32)
    for cid in range(NC):
        ka, kb = (0, 1) if cid < 4 else (2, 3)
        q = cid % 4
        o = results[cid]["out"]
        full[ka, 1 + q * PPC:1 + (q + 1) * PPC] = o[0, 1:]
        full[kb, 1 + q * PPC:1 + (q + 1) * PPC] = o[1, 1:]
        if q == 0:
            full[ka, 0] = o[0, 0]
            full[kb, 0] = o[1, 0]
    return full


def kernel(X, Y, bandwidths, perms):
    nc = _get_nc()
    in_maps = _host_prep(X, Y, bandwidths, perms)
    res = bass_utils.run_bass_kernel_spmd(nc, in_maps, list(range(NC)))
    return _merge(res.results)


# revision 34
# speedup vs baseline: 1.2226x; 1.0268x over previous
"""Trainium2 Bass kernel for the 4-kernel MMD permutation test (nn_DUAL_78237124264373).

Sharding: 8 cores = 2 kernel-pairs x 4 permutation quarters. Core c<4 computes
kernels (0,1) [gaussian, laplacian] for perms [50*(c%4), 50*(c%4)+50); core
c>=4 the same for kernels (2,3). The host merges the [2, 1+50] per-core
outputs, so each core only ever evaluates TWO kernel matrices and the
activation-table sequence is exactly EXP (pre-warmed) -> SQRT -> EXP: the
swap points are pinned with zero-valued bias/scale tokens that data-depend
on the previous block's last op, so the Tile scheduler cannot interleave.

Per-core pipeline (slot a = gaussian, slot b = laplacian):
  d2 = L^T R on the PE in f32r (L = [Zt; 1], R = [-2 Zt; sq + B]), 12 PSUM
  pieces (6 row tiles x 512+256) in a 4-deep PSUM pool. Slot-a K = exp(ga*d2)
  straight out of PSUM with its M0 = A_aug K chunk matmul right behind, while
  the DVE lands clamped d2 in SBUF for the sqrt block. After the swap back to
  EXP, slot-b K = exp(lb*dist) runs chunk-by-chunk with M0 trailing. Each
  slot's row stats (aKa, aK1, colA), U_b vector and PE transpose into the
  partition-0 assembly row run as soon as that slot's M0 completes, so slot
  a's tail hides under the SQRT/slot-b window. U_b = KAP*(aKa - aK1) +
  W_corr @ e + (2/c2)*t + ck, with pair sums t reduced from host-gathered
  16-wide partial squares and e the K0[j, 384+j] stripe (3 extra pair
  blocks). The final scalar assembly (U, ck) happens on partition 0.
"""

import sys

import numpy as np

if "/opt/trn_rl_repo" not in sys.path:
    sys.path.insert(0, "/opt/trn_rl_repo")

import ml_dtypes

import concourse.bacc as bacc
import concourse.bass as bass
import concourse.mybir as mybir
import concourse.tile as tile
from concourse import bass_utils

N = 384
NM = 768
D = 64
NPER = 200
NC = 8
PPC = 50                      # perms per core
ROWS = PPC + 2                # + X-identity + Y-identity rows
NBLK = 3 * PPC + 3            # pair blocks of 128: 50 perms x 3 + stripe x 3
BIAS = 1e-3                   # keeps d2 > 0 under f32r rounding (see d0c)
C1 = float(N * (N - 1))
C2 = float(N * N)
KAP = np.float32(2.0 / C1 + 2.0 / C2)
CB1 = np.float32(1.0 / C1 + 2.0 / C2)
CB2 = np.float32(1.0 / C1)
TCO = np.float32(2.0 / C2)
IC1 = np.float32(1.0 / C1)
IC2 = np.float32(1.0 / C2)

F32 = mybir.dt.float32
F32R = mybir.dt.float32r
BF16 = mybir.dt.bfloat16
AF = mybir.ActivationFunctionType
ALU = mybir.AluOpType


def _build():
    nc = bacc.Bacc("TRN2", target_bir_lowering=False, debug=False)
    with tile.TileContext(nc) as tc:
        with tc.tile_pool(name="dram", bufs=1, space="DRAM") as dram, \
             tc.tile_pool(name="io", bufs=1) as io, \
             tc.tile_pool(name="big", bufs=1) as big, \
             tc.tile_pool(name="scr", bufs=1) as scr, \
             tc.tile_pool(name="sml", bufs=1) as sml:

            def din(name, shape, dt=F32):
                return dram.tile(shape, dt, kind="ExternalInput", name=name,
                                 uniquify=False)

            zlr_d = din("zlr", [D + 1, 2 * NM], F32R)
            bfp_d = din("bfp", [128, NBLK * 4 + NM + 6 * D + 3 * D], BF16)
            fsp_d = din("fsp", [128, 32 + 64], F32)
            out_d = dram.tile([2, 1 + PPC], F32, kind="ExternalOutput",
                              name="out", uniquify=False)

            # ---- input DMAs; zlr is [R | L] so the first piece lands first
            fsp = io.tile([128, 32 + 64], F32, name="fsp_sb")
            zlr = io.tile([D + 1, 2 * NM], F32R, name="zlr_sb")
            nc.sync.dma_start(out=zlr[:, 0:640], in_=zlr_d[:, 0:640])
            nc.scalar.dma_start(out=fsp[:], in_=fsp_d[:])
            nc.gpsimd.dma_start(out=zlr[:, 640:], in_=zlr_d[:, 640:])
            bfp = io.tile([128, NBLK * 4 + NM + 6 * D + 3 * D], BF16,
                          name="bfp_sb")
            nc.sync.dma_start(out=bfp[:], in_=bfp_d[:])

            zr1 = zlr[:, 0:512]
            zl0 = zlr[:, 512:640]
            zr2 = zlr[:, 640:896]
            zlrest = zlr[:, 896:2 * NM]
            idm = fsp[:, 32:96]
            psq = bfp[:, 0:NBLK * 4]
            PB = NBLK * 4
            astk = bfp[:, PB:PB + NM]                # A rows at 0-51 / 64-115
            atp = bfp[:, PB + NM:PB + NM + 6 * D]    # A^T chunks, 64-padded
            wct = bfp[:, PB + NM + 6 * D:PB + NM + 9 * D]
            sqc = fsp[:, 0:6]                        # sq columns per row tile
            gbias = fsp[:, 6:12]                     # ga * sq per row tile
            ga = fsp[:, 12:13]
            lb = fsp[:, 13:14]
            zero = fsp[:, 14:15]
            aux4i = fsp[0:1, 16:18]                  # 768*d0c*IC1 per slot

            ones = io.tile([128, 1], F32, name="ones_sb")
            nc.vector.memset(ones[:], 1.0)
            onesb = io.tile([128, 1], BF16, name="onesb_sb")
            nc.vector.memset(onesb[:], 1.0)

            d2sb = big.tile([128, 6 * NM], F32, name="d2sb")
            dist = big.tile([128, 6 * NM], F32, name="dist_sb")
            kta = big.tile([128, 6 * NM], BF16, name="kta")
            ktb = big.tile([128, 6 * NM], BF16, name="ktb")
            M0sb = big.tile([128, NM], F32, name="M0sb")
            sA = scr.tile([128, N], F32, name="sA")
            sB = scr.tile([128, NM], F32, name="sB")
            pair2 = sml.tile([128, NBLK], F32, name="pair2")
            arow = sml.tile([128, 1], F32, name="arow")
            colA = sml.tile([128, 1], F32, name="colA")
            q0c = sml.tile([128, 1], F32, name="q0c")
            pack = sml.tile([128, 4], F32, name="pack")
            frow = sml.tile([1, 512], F32, name="frow")
            ubc = sml.tile([1, 2 * (PPC + 1)], F32, name="ubc")

            # warm the EXP activation table while DMAs are in flight
            warm = sml.tile([128, 1], F32, name="warm")
            nc.scalar.activation(warm[0:1, :], ones[0:1, :], AF.Exp,
                                 bias=0.0, scale=1.0)

            # ---- pair d2: reduce the host 16-wide partial squares ----
            psq3 = psq.rearrange("p (b d) -> p b d", d=4)
            nc.vector.tensor_reduce(pair2[:], psq3[:],
                                    axis=mybir.AxisListType.X, op=ALU.add)

            # ---- d2 phase: f32r matmuls, one [128,768] PSUM tile per row
            # tile, 3 deep; slot-a exp from PSUM; DVE lands d2 for sqrt ----
            with tc.tile_pool(name="psA", bufs=4, space="PSUM") as psA:
                for r in range(6):
                    lhs = (zl0 if r == 0 else
                           zlrest[:, 128 * (r - 1):128 * r])
                    ps_d2 = psA.tile([128, NM], F32, tag="d2",
                                     name=f"ps_d2_{r}")
                    nc.tensor.matmul(ps_d2[:, 0:512], lhs, zr1[:],
                                     start=True, stop=True)
                    nc.tensor.matmul(ps_d2[:, 512:NM], lhs, zr2[:],
                                     start=True, stop=True)
                    sl = slice(NM * r, NM * (r + 1))
                    nc.scalar.activation(kta[:, sl], ps_d2[:], AF.Exp,
                                         scale=ga, bias=gbias[:, r:r + 1])
                    nc.vector.tensor_scalar(
                        out=d2sb[:, sl], in0=ps_d2[:],
                        scalar1=sqc[:, r:r + 1], scalar2=0.0,
                        op0=ALU.add, op1=ALU.max)


            with tc.tile_pool(name="psB", bufs=1, space="PSUM") as psB, \
                 tc.tile_pool(name="psC", bufs=1, space="PSUM") as psC:

                ps_m = psB.tile([128, NM], F32, name="ps_m")
                ps_tc = psC.tile([128, 2], F32, name="ps_tc")
                ps_t = ps_tc[:, 0:1]
                ps_corr = ps_tc[:, 1:2]
                ps_row = ps_m[0:1, 0:512]

                # slot-a pair exp rides the warm EXP table before the swap
                pea = sml.tile([128, NBLK], BF16, name="pea")
                nc.scalar.activation(pea[:], pair2[:], AF.Exp,
                                     bias=zero, scale=ga)
                # zb: zero bias that depends on the last EXP-block op, so
                # the scheduler cannot move the SQRT block earlier
                zb1 = sml.tile([128, 1], F32, name="zb1")
                nc.gpsimd.tensor_scalar(
                    out=zb1[:], in0=kta[:, 6 * NM - 1:6 * NM],
                    scalar1=0.0, scalar2=0.0, op0=ALU.mult, op1=ALU.add)
                zb = sml.tile([128, 1], F32, name="zb")
                nc.gpsimd.tensor_tensor(out=zb[:], in0=zb1[:],
                                        in1=pea[:, 0:1], op=ALU.mult)

                # slot-a M0 runs after the whole d2 phase (in the idle PE
                # sqrt window): interleaved, each d2 matmul would chain
                # behind the previous exp in the PE's in-order queue. The
                # zb-gated copy of atp pins the ordering.
                atp2 = scr.tile([128, 6 * D], BF16, name="atp2")
                nc.gpsimd.tensor_scalar(
                    out=atp2[:], in0=atp[:], scalar1=1.0, scalar2=zb[:],
                    op0=ALU.mult, op1=ALU.add)
                for r in range(6):
                    for fs in (slice(0, 512), slice(512, NM)):
                        nc.tensor.matmul(ps_m[0:64, fs],
                                         atp2[:, D * r:D * r + 64],
                                         kta[:, NM * r + fs.start:
                                              NM * r + fs.stop],
                                         start=(r == 0), stop=(r == 5),
                                         tile_position=(0, 0),
                                         skip_group_check=True)

                # ---- swap to SQRT: dist halves, then pair dist ----
                for h in range(2):
                    hs = slice(3 * NM * h, 3 * NM * (h + 1))
                    nc.scalar.activation(dist[:, hs], d2sb[:, hs], AF.Sqrt,
                                         bias=zb, scale=1.0)
                zb2 = sml.tile([128, 1], F32, name="zb2")
                nc.gpsimd.tensor_scalar(
                    out=zb2[:], in0=dist[:, 3 * NM - 1:3 * NM],
                    scalar1=0.0, scalar2=0.0, op0=ALU.mult, op1=ALU.add)
                pdist = sml.tile([128, NBLK], F32, name="pdist")
                nc.scalar.activation(pdist[:], pair2[:], AF.Sqrt,
                                     bias=zb2, scale=1.0)
                # lbt == lb, but depends on the last SQRT-block op
                lbt = sml.tile([128, 1], F32, name="lbt")
                nc.gpsimd.tensor_scalar(
                    out=lbt[:], in0=pdist[:, 0:1], scalar1=0.0,
                    scalar2=lb, op0=ALU.mult, op1=ALU.add)

                def slot_tail(i, pe):
                    """Pair sums, corrections, row stats, ubv and the PE
                    transpose for slot i; runs as soon as its M0 stops."""
                    pt = slice(64 * i, 64 * i + 64)
                    # t3: per-perm 3-block partial sums; group PPC holds the
                    # stripe so t[50] = sum(e) lands in ps_t for free
                    pe3 = pe.rearrange("p (g t) -> p g t", t=3)
                    t3 = sml.tile([128, 64], BF16, name=f"t3_{i}")
                    nc.vector.memset(t3[:, PPC + 1:64], 0.0)
                    with nc.allow_low_precision(reason="3-wide bf16 sum"):
                        nc.vector.tensor_reduce(t3[:, 0:PPC + 1], pe3[:],
                                                axis=mybir.AxisListType.X,
                                                op=ALU.add)
                    nc.tensor.matmul(ps_t[pt, :], t3[:], onesb[:],
                                     start=True, stop=True,
                                     tile_position=(0, 64 * i),
                                     skip_group_check=True)
                    for c in range(3):
                        nc.tensor.matmul(
                            ps_corr[pt, :], wct[:, D * c:D * c + 64],
                            pe[:, 3 * PPC + c:3 * PPC + c + 1],
                            start=(c == 0), stop=(c == 2),
                            tile_position=(0, 64 * i),
                            skip_group_check=True)
                    # row stats off this slot's half of ps_m; slot b uses
                    # the (idle) Scalar engine for the copy+rowsum so only
                    # sA rides the DVE tail; q0 reads PSUM directly
                    nc.vector.scalar_tensor_tensor(
                        out=sB[pt, :], in0=ps_m[pt, :], scalar=1.0,
                        in1=astk[pt, :], op0=ALU.mult, op1=ALU.mult,
                        accum_out=pack[pt, 1:2])
                    nc.vector.tensor_scalar(
                        out=M0sb[pt, :], in0=ps_m[pt, :], scalar1=1.0,
                        scalar2=0.0, op0=ALU.mult, op1=ALU.add,
                        accum_out=pack[pt, 2:3])
                    # ubv = KAP*(q0 - arow) + corr + TCO*t into pack col 0;
                    # q0 / arow / t stay in cols 1-3 for the transpose
                    nc.vector.tensor_tensor(out=pack[pt, 0:1],
                                            in0=pack[pt, 1:2],
                                            in1=pack[pt, 2:3],
                                            op=ALU.subtract)
                    nc.vector.scalar_tensor_tensor(
                        out=pack[pt, 0:1], in0=pack[pt, 0:1],
                        scalar=float(KAP), in1=ps_corr[pt, :],
                        op0=ALU.mult, op1=ALU.add)
                    nc.vector.scalar_tensor_tensor(
                        out=pack[pt, 0:1], in0=ps_t[pt, :],
                        scalar=float(TCO), in1=pack[pt, 0:1],
                        op0=ALU.mult, op1=ALU.add)
                    nc.vector.tensor_copy(pack[pt, 3:4], ps_t[pt, :])
                    # transpose the 4 pack columns into the partition-0 row
                    # (ps_m bank 0 is free again: stats above read it first)
                    for k in range(4):
                        nc.tensor.matmul(
                            ps_row[0:1,
                                   128 * k + 64 * i:128 * k + 64 * i + 64],
                            pack[pt, k:k + 1], idm[pt, :],
                            is_transpose=True, start=True, stop=True,
                            tile_position=(64 * i, 0),
                            skip_group_check=True)
                    s0 = ps_row[0:1, 64 * i:64 * i + 1]
                    f0 = frow[0:1, 64 * i:64 * i + 1]
                    nc.vector.tensor_copy(
                        bass.AP(f0.tensor, f0.offset,
                                [f0.ap[0], [128, 4], [1, 64]]),
                        bass.AP(s0.tensor, s0.offset,
                                [s0.ap[0], [128, 4], [1, 64]]))

                    # per-slot partition-0 assembly: U, ck, output row
                    def sv(row, col, *rest):
                        ap = frow[0:1, 128 * row + 64 * i + col:
                                  128 * row + 64 * i + col + 1]
                        return bass.AP(ap.tensor, ap.offset,
                                       [ap.ap[0], *(rest or ([1, 1],))])

                    XXv = sv(1, PPC)
                    YYv = sv(1, PPC + 1)
                    aXv = sv(2, PPC)
                    sev = sv(3, PPC)
                    s0q = sv(2, PPC, [1, 2])
                    s0t = sml.tile([1, 2], F32, name=f"s0t{i}")
                    nc.vector.tensor_reduce(s0t[0:1, 0:1], s0q,
                                            axis=mybir.AxisListType.X,
                                            op=ALU.add)
                    ck = sml.tile([1, 2], F32, name=f"ck{i}")
                    nc.vector.tensor_tensor(out=ck[0:1, 0:1],
                                            in0=s0t[0:1, 0:1], in1=sev,
                                            op=ALU.subtract)
                    nc.vector.scalar_tensor_tensor(
                        out=ck[0:1, 0:1], in0=ck[0:1, 0:1],
                        scalar=float(IC1), in1=aux4i[0:1, i:i + 1],
                        op0=ALU.mult, op1=ALU.subtract)
                    u1 = sml.tile([1, 2], F32, name=f"u1{i}")
                    nc.vector.tensor_tensor(out=u1[0:1, 0:1], in0=XXv,
                                            in1=YYv, op=ALU.add)
                    nc.vector.scalar_tensor_tensor(
                        out=u1[0:1, 0:1], in0=u1[0:1, 0:1],
                        scalar=float(IC1), in1=aux4i[0:1, i:i + 1],
                        op0=ALU.mult, op1=ALU.subtract)
                    u2 = sml.tile([1, 2], F32, name=f"u2{i}")
                    nc.vector.tensor_tensor(out=u2[0:1, 0:1], in0=aXv,
                                            in1=XXv, op=ALU.subtract)
                    nc.vector.tensor_tensor(out=u2[0:1, 0:1],
                                            in0=u2[0:1, 0:1], in1=sev,
                                            op=ALU.subtract)
                    nc.vector.scalar_tensor_tensor(
                        out=ubc[0:1, (PPC + 1) * i:(PPC + 1) * i + 1],
                        in0=u2[0:1, 0:1], scalar=float(-2.0 * IC2),
                        in1=u1[0:1, 0:1], op0=ALU.mult, op1=ALU.add)
                    ub0 = frow[0:1, 64 * i:64 * i + 1]
                    ub_src = bass.AP(ub0.tensor, ub0.offset,
                                     [ub0.ap[0], [1, PPC]])
                    ckap = ck[0:1, 0:1]
                    ck_b = bass.AP(ckap.tensor, ckap.offset,
                                   [ckap.ap[0], [0, PPC]])
                    nc.vector.tensor_tensor(
                        out=ubc[0:1, (PPC + 1) * i + 1:(PPC + 1) * (i + 1)],
                        in0=ub_src, in1=ck_b, op=ALU.add)

                slot_tail(0, pea)

                # ---- swap back to EXP: slot-b K chunks + pair exp ----
                for c in range(3):
                    cs = slice(2 * NM * c, 2 * NM * (c + 1))
                    nc.scalar.activation(ktb[:, cs], dist[:, cs], AF.Exp,
                                         scale=lbt, bias=zero)
                    for r in (2 * c, 2 * c + 1):
                        for fs in (slice(0, 512), slice(512, NM)):
                            nc.tensor.matmul(ps_m[64:128, fs],
                                             atp[:, D * r:D * r + 64],
                                             ktb[:, NM * r + fs.start:
                                                  NM * r + fs.stop],
                                             start=(r == 0), stop=(r == 5),
                                             tile_position=(0, 64),
                                             skip_group_check=True)
                peb = sml.tile([128, NBLK], BF16, name="peb")
                nc.scalar.activation(peb[:], pdist[:], AF.Exp,
                                     bias=zero, scale=lbt)

                slot_tail(1, peb)

                # output row was assembled per slot in slot_tail
                nc.sync.dma_start(
                    out=out_d[:, :],
                    in_=ubc[0:1, :].rearrange("o (k p) -> o k p",
                                              p=PPC + 1))

    nc.compile()
    return nc


def _host_prep(X, Y, bandwidths, perms):
    X = np.ascontiguousarray(X, np.float32)
    Y = np.ascontiguousarray(Y, np.float32)
    perms = np.ascontiguousarray(perms, np.int32)
    Zf = np.concatenate([X, Y], 0)
    Zt = Zf.T.astype(np.float32)
    sq = (Zf.astype(np.float64) ** 2).sum(1).astype(np.float32)
    b = np.asarray(bandwidths, np.float64)

    zlr = np.zeros((D + 1, 2 * NM), np.float32)
    R = np.concatenate([-2.0 * Zt, (sq + BIAS)[None, :]], 0)
    L = np.concatenate([Zt, np.ones((1, NM), np.float32)], 0)
    zlr[:, 0:512] = R[:, 0:512]
    zlr[:, 512:640] = L[:, 0:128]
    zlr[:, 640:896] = R[:, 512:768]
    zlr[:, 896:] = L[:, 128:768]

    idm = np.tile(np.eye(64, dtype=np.float32), (2, 1))

    maps = []
    for cid in range(NC):
        ka, kb = (0, 1) if cid < 4 else (2, 3)
        q = cid % 4
        pm = perms[q * PPC:(q + 1) * PPC]

        A = np.zeros((ROWS, NM), np.float32)
        A[np.arange(PPC)[:, None], pm[:, :N]] = 1
        A[PPC, :N] = 1
        A[PPC + 1, N:] = 1
        astk = np.zeros((128, NM), np.float32)
        astk[0:ROWS] = A
        astk[64:64 + ROWS] = A
        atp = np.zeros((128, 6 * D), np.float32)
        for c in range(6):
            atp[:, D * c:D * c + ROWS] = A[:, 128 * c:128 * (c + 1)].T
        A1 = A[:PPC, :N]
        A2 = A[:PPC, N:]
        Wc = (-KAP * (A1 * A2) + CB1 * A1 + CB2 * A2).astype(np.float32)
        wct = np.zeros((128, 3 * D), np.float32)
        for c in range(3):
            wct[:, D * c:D * c + PPC] = Wc[:, 128 * c:128 * (c + 1)].T
        bfp = np.zeros((128, NBLK * 4 + NM + 6 * D + 3 * D), np.float32)
        PB = NBLK * 4
        bfp[:, PB:PB + NM] = astk
        bfp[:, PB + NM:PB + NM + 6 * D] = atp
        bfp[:, PB + NM + 6 * D:PB + NM + 9 * D] = wct

        # pair partial squares: perm p pair j at lane (384p+j)%128, block
        # (384p+j)//128, 4 groups of 16 dims. Stripe pairs (j, 384+j) fill
        # blocks 3*PPC..3*PPC+2; stripe hits inside perm rows get a huge
        # sentinel so exp -> 0 (the zeroed K stripe).
        pX = pm[:, :N].astype(np.int64).ravel()
        pY = pm[:, N:].astype(np.int64).ravel()
        pdv = (Zf[pX] - Zf[pY]).astype(np.float32) ** 2
        psq = pdv.reshape(-1, 4, 16).sum(2) + np.float32(BIAS / 4)
        psq[pY == pX + N] = 1e6
        sdv = (Zf[:N] - Zf[N:]).astype(np.float32) ** 2
        psq = np.concatenate(
            [psq, sdv.reshape(-1, 4, 16).sum(2) + np.float32(BIAS / 4)], 0)
        psq = psq.reshape(NBLK, 128, 4).transpose(1, 0, 2).reshape(128, -1)
        bfp[:, 0:PB] = psq

        fsp = np.zeros((128, 32 + 64), np.float32)
        fsp[:, 32:96] = idm
        ga = np.float32(-1.0 / (b[ka] * b[ka]))
        lb = np.float32(-1.0 / b[kb])
        sqcols = sq.reshape(6, 128).T
        fsp[:, 0:6] = sqcols
        fsp[:, 6:12] = ga * sqcols
        fsp[:, 12] = ga
        fsp[:, 13] = lb
        fsp[:, 14] = 0.0
        d0a = np.exp(-BIAS / (b[ka] * b[ka]))
        d0b = np.exp(-np.sqrt(BIAS) / b[kb])
        fsp[0, 16] = np.float32(NM * d0a * IC1)
        fsp[0, 17] = np.float32(NM * d0b * IC1)

        maps.append(dict(zlr=zlr, bfp=bfp.astype(ml_dtypes.bfloat16),
                         fsp=fsp))
    return maps


_NC_CACHE = None


def _get_nc():
    global _NC_CACHE
    if _NC_CACHE is None:
        _NC_CACHE = _build()
    return _NC_CACHE


def _merge(results):
    full = np.zeros((4, 1 + NPER), np.float32)
    for cid in range(NC):
        ka, kb = (0, 1) if cid < 4 else (2, 3)
        q = cid % 4
        o = results[cid]["out"]
        full[ka, 1 + q * PPC:1 + (q + 1) * PPC] = o[0, 1:]
        full[kb, 1 + q * PPC:1 + (q + 1) * PPC] = o[1, 1:]
        if q == 0:
            full[ka, 0] = o[0, 0]
            full[kb, 0] = o[1, 0]
    return full


def kernel(X, Y, bandwidths, perms):
    nc = _get_nc()
    in_maps = _host_prep(X, Y, bandwidths, perms)
    res = bass_utils.run_bass_kernel_spmd(nc, in_maps, list(range(NC)))
    return _merge(res.results)
